# revision 39
# baseline (speedup 1.0000x reference)
"""DiffNet++ (GATv2 diffusion + gamma gating + dot-product prediction) on 8
Trainium2 NeuronCores via Bass/Tile.

Strategy (dst-range edge sharding, one SPMD program):
  - Users/items row-sharded equally: users 98 tiles (12544 rows)/core, items 49
    tiles (6272 rows)/core. Each GAT edge belongs to the core owning its dst.
  - Per core, edges are grouped by dst "window" (128 rows) and bucket-sorted by
    src-table bank (dma_gather int16 index => 32768-row banks). The padded slot
    structure is the max over cores, so one program serves all cores.
  - Segment softmax without max subtraction (logits ~1e-2): out[v] =
    (sum_e exp(e) fs[src]) / (sum_e exp(e)), accumulated via one-hot matmuls
    into PSUM windows; per-dst divide afterwards.
  - fs[src] rows: dma_gather (batched indirect DMA) spread round-robin over 4
    SWDGE queues (gathers are descriptor-rate-bound; one queue caps in-flight
    descriptors). fd[dst]: expanded from the contiguous dst window by one-hot
    fp16 matmuls; one-hots built batched (1 DVE is_equal per 4 sub-tiles for
    expansion, 1 per block for scatter); scatter matmuls in fp16.
  - The 3 GATs of a layer are emitted block-interleaved in one phase/pool set
    so gather DMA streams overlap the other GATs' DVE/PE compute.
  - Projections row-sharded + AllGather (Shared outputs = fast collective
    path); updated embeddings AllGather/layer (Shared).
  - Prediction: hu/hi concat tables in fp16 padded to 256 cols; gather both
    sides per edge; fused multiply-reduce dots.
"""
import sys

sys.path.insert(0, "/opt/trn_rl_repo")

from contextlib import ExitStack

import numpy as np
import ml_dtypes

import concourse.bass as bass
import concourse.tile as tile
from concourse import bacc, mybir
from concourse.bass_utils import run_bass_kernel_spmd
from concourse.masks import make_identity

N_CORES = 8
P = 128
BANK = 32768
GAT_SLOPE = 0.2
MLP_SLOPE = 0.01
F16 = mybir.dt.float16
F32 = mybir.dt.float32
I16 = mybir.dt.int16
NPF16 = np.dtype("float16")

Alu = mybir.AluOpType
Act = mybir.ActivationFunctionType


def _ceil(a, b):
    return -(-a // b)


# ---------------------------------------------------------------------------
# host-side preprocessing
# ---------------------------------------------------------------------------

class GatStruct:
    """Canonical (core-uniform) structure for one GAT graph's edges."""

    def __init__(self, name, src, dst, table_rows, shard_tiles):
        self.name = name
        self.nb = _ceil(table_rows, BANK)
        self.shard_tiles = shard_tiles
        S = shard_tiles * P
        self.S = S

        core = np.minimum(dst // S, N_CORES - 1)
        win = (dst - core * S) // P
        bank = src // BANK

        cnt = np.zeros((N_CORES, shard_tiles, self.nb), dtype=np.int64)
        np.add.at(cnt, (core, win, bank), 1)
        self.Kb = [max(1, int(_ceil(int(cnt[:, :, b].max()), P)))
                   for b in range(self.nb)]
        self.K = sum(self.Kb)
        self.WB = max(1, min(7, 80 // self.K))
        self.blocks = []
        t = shard_tiles
        while t > 0:
            wbi = min(self.WB, t)
            self.blocks.append(wbi)
            t -= wbi
        self.G_total = shard_tiles * self.K  # sub-tiles per core overall
        self.total_cols = self.G_total * P // 16

        order = np.lexsort((bank, win, core))
        src_s = src[order]
        dst_s = dst[order]
        core_s = core[order]
        win_s = win[order]
        bank_s = bank[order]

        self.idx16 = []
        self.dlc = []
        self.dlr = []
        for c in range(N_CORES):
            sel = core_s == c
            csrc = src_s[sel]
            cdst = dst_s[sel]
            cwin = win_s[sel]
            cbank = bank_s[sel]
            key = cwin.astype(np.int64) * self.nb + cbank
            ids = np.zeros((self.G_total * P,), dtype=np.int16)
            dl = np.full((self.G_total * P,), -1.0, dtype=NPF16)
            # slot layout: per block: [bank b: [window wo: Kb[b]*128 slots]]
            slot0 = 0
            w_base = 0
            for wbi in self.blocks:
                for b in range(self.nb):
                    for wo in range(wbi):
                        w = w_base + wo
                        e0 = np.searchsorted(key, w * self.nb + b, "left")
                        e1 = np.searchsorted(key, w * self.nb + b, "right")
                        n = e1 - e0
                        nsw = self.Kb[b] * P
                        assert n <= nsw, (name, c, w, b, n, nsw)
                        ids[slot0:slot0 + n] = (csrc[e0:e1] - b * BANK).astype(np.int16)
                        dl[slot0:slot0 + n] = (cdst[e0:e1] - (c * S + w * P)).astype(NPF16)
                        slot0 += nsw
                w_base += wbi
            assert slot0 == self.G_total * P
            cols = self.total_cols
            a = np.empty((16, cols), dtype=np.int16)
            j = np.arange(self.G_total * P)
            a[j % 16, j // 16] = ids
            self.idx16.append(np.tile(a, (8, 1)))
            self.dlc.append(np.ascontiguousarray(
                dl.reshape(self.G_total, P).T))          # [128, G_total]
            self.dlr.append(dl.reshape(1, -1).copy())    # [1, G_total*128]


class PredStruct:
    """Canonical structure for prediction edges (pos+neg concatenated)."""

    def __init__(self, src, dst, u_rows, i_rows, block_edges):
        E = len(src)
        assert E % N_CORES == 0
        per_core = E // N_CORES
        self.per_core = per_core
        self.nbu = _ceil(u_rows, BANK)
        self.nbi = _ceil(i_rows, BANK)
        self.n_blocks = _ceil(per_core, block_edges)
        pairs = [(u_, i_) for u_ in range(self.nbu) for i_ in range(self.nbi)]
        self.pairs = pairs

        core = np.arange(E) // per_core
        blk = (np.arange(E) % per_core) // block_edges
        ub = src // BANK
        ib = dst // BANK
        cnt = np.zeros((N_CORES, self.n_blocks, self.nbu, self.nbi), dtype=np.int64)
        np.add.at(cnt, (core, blk, ub, ib), 1)
        self.Kp = {pq: max(1, int(_ceil(int(cnt[:, :, pq[0], pq[1]].max()), P)))
                   for pq in pairs}
        self.G_blk = sum(self.Kp[pq] for pq in pairs)
        self.G_total = self.G_blk * self.n_blocks

        self.idxu = []
        self.idxi = []
        self.slotmap = []
        for c in range(N_CORES):
            lo = c * per_core
            cs = src[lo:lo + per_core]
            cd = dst[lo:lo + per_core]
            iu = np.zeros((self.G_total * P,), dtype=np.int16)
            ii = np.zeros((self.G_total * P,), dtype=np.int16)
            smap = np.full((self.G_total * P,), -1, dtype=np.int64)
            for bi in range(self.n_blocks):
                b0 = bi * block_edges
                b1 = min(b0 + block_edges, per_core)
                bs, bd = cs[b0:b1], cd[b0:b1]
                bub, bib = bs // BANK, bd // BANK
                key = bub.astype(np.int64) * self.nbi + bib
                ordk = np.argsort(key, kind="stable")
                keys = key[ordk]
                off = bi * self.G_blk * P
                for pq in pairs:
                    kv = pq[0] * self.nbi + pq[1]
                    e0 = np.searchsorted(keys, kv, "left")
                    e1 = np.searchsorted(keys, kv, "right")
                    n = e1 - e0
                    npad = self.Kp[pq] * P
                    assert n <= npad
                    sel2 = ordk[e0:e1]
                    iu[off:off + n] = (bs[sel2] - pq[0] * BANK).astype(np.int16)
                    ii[off:off + n] = (bd[sel2] - pq[1] * BANK).astype(np.int16)
                    smap[off:off + n] = lo + b0 + sel2
                    off += npad
            cols = self.G_total * P // 16
            j = np.arange(self.G_total * P)
            au = np.empty((16, cols), dtype=np.int16)
            au[j % 16, j // 16] = iu
            ai = np.empty((16, cols), dtype=np.int16)
            ai[j % 16, j // 16] = ii
            self.idxu.append(np.tile(au, (8, 1)))
            self.idxi.append(np.tile(ai, (8, 1)))
            self.slotmap.append(smap)


# ---------------------------------------------------------------------------
# program builder
# ---------------------------------------------------------------------------

def bench_pjrt(nc, in_maps, iters=3):
    """Time steady-state executions of the compiled program on the 8 cores.

    Rebuilds the PJRT callable without donation, uploads inputs once, then
    times back-to-back executions."""
    import time as _time
    import jax
    from jax.sharding import Mesh, PartitionSpec
    from jax.experimental.shard_map import shard_map
    from concourse import bass2jax
    from concourse import mybir as _mb

    bass2jax.install_neuronx_cc_hook()
    partition_name = (nc.partition_id_tensor.name
                      if nc.partition_id_tensor else None)
    in_names, out_names, out_avals = [], [], []
    for alloc in nc.m.functions[0].allocations:
        if not isinstance(alloc, _mb.MemoryLocationSet):
            continue
        name = alloc.memorylocations[0].name
        if alloc.kind == "ExternalInput":
            if name != partition_name:
                in_names.append(name)
        elif alloc.kind == "ExternalOutput":
            out_names.append(name)
            out_avals.append(jax.core.ShapedArray(
                tuple(alloc.tensor_shape), _mb.dt.np(alloc.dtype)))
    n_params = len(in_names)
    zero_outs = [np.zeros(a.shape, a.dtype) for a in out_avals]
    all_names = in_names + out_names
    if partition_name is not None:
        all_names = all_names + [partition_name]

    def _body(*args):
        operands = list(args)
        if partition_name is not None:
            operands.append(bass2jax.partition_id_tensor())
        return tuple(bass2jax._bass_exec_p.bind(
            *operands, out_avals=tuple(out_avals),
            in_names=tuple(all_names), out_names=tuple(out_names),
            lowering_input_output_aliases=(), sim_require_finite=True,
            sim_require_nnan=True, nc=nc))

    devices = jax.devices()[:N_CORES]
    mesh = Mesh(np.asarray(devices), ("core",))
    nspec = n_params + len(out_names)
    f = jax.jit(shard_map(_body, mesh=mesh,
                          in_specs=(PartitionSpec("core"),) * nspec,
                          out_specs=(PartitionSpec("core"),) * len(out_names),
                          check_rep=False), keep_unused=True)
    from jax.sharding import NamedSharding
    sh = NamedSharding(mesh, PartitionSpec("core"))
    concat_in = [np.concatenate([np.asarray(m[nm]) for m in in_maps], axis=0)
                 for nm in in_names]
    concat_in += [np.concatenate([z] * N_CORES, axis=0) for z in zero_outs]
    dev_in = [jax.device_put(x, sh) for x in concat_in]
    times = []
    for i in range(iters):
        t0 = _time.time()
        outs = f(*dev_in)
        jax.block_until_ready(outs)
        times.append(_time.time() - t0)
    print(f"[bench] iter times: {[f'{t*1e3:.2f}ms' for t in times]}")
    # pipelined: issue PIPE calls back-to-back, block once
    PIPE = int(os.environ.get("KPIPE", "128")) if (os := __import__("os")) else 128
    outs = [f(*dev_in) for _ in range(2)]
    jax.block_until_ready(outs)  # warm
    t0 = _time.time()
    outs = [f(*dev_in) for _ in range(PIPE)]
    jax.block_until_ready(outs)
    piped = (_time.time() - t0) / PIPE
    print(f"[bench] pipelined per-iter: {piped*1e3:.2f}ms")
    return min(min(times[1:]) if len(times) > 1 else times[0], piped)


def build_program(hp):
    U, I, D, L = hp["U"], hp["I"], hp["D"], hp["L"]
    UT, IT = hp["UT"], hp["IT"]
    US, IS = UT * P, IT * P
    UPAD, IPAD = US * N_CORES, IS * N_CORES
    rate, rb, tr = hp["rate"], hp["rb"], hp["tr"]
    pred = hp["pred"]
    PD = hp["PD"]
    CD = D * (L + 1)

    nc = bacc.Bacc("TRN2", target_bir_lowering=False, debug=False,
                   num_devices=N_CORES, num_swdge_queues=4)

    def inp(name, shape, dt):
        return nc.dram_tensor(name, list(shape), dt, kind="ExternalInput")

    user_emb = inp("user_emb", [UPAD, D], F32)       # full, padded
    item_emb = inp("item_emb", [IPAD, D], F32)
    u_shard0 = inp("u_shard0", [US, D], F32)         # per-core slice
    it_shard0 = inp("it_shard0", [IS, D], F32)
    wu = inp("wu", [D, L * 4 * D], F32)
    bu = inp("bu", [P, L * 4 * D], F32)
    wi = inp("wi", [D, L * 2 * D], F32)
    bi_ = inp("bi", [P, L * 2 * D], F32)
    a_in = {g.name: inp(f"a_{g.name}", [P, L * D], F32) for g in (rate, rb, tr)}
    w1 = inp("w1", [2 * D, L * 2 * D], F32)
    b1 = inp("b1", [P, L * 2 * D], F32)
    w2 = inp("w2", [P, L * 2 * D], F32)
    b2 = inp("b2", [P, L * 2], F32)
    iota_m_in = inp("iota_m", [P, P], F16)
    iota_c_in = inp("iota_c", [P, 1], F16)
    ones_r_in = inp("ones_r", [1, P], F16)

    g_in = {}
    for g in (rate, rb, tr):
        g_in[g.name] = {
            "idx": inp(f"{g.name}_idx", list(g.idx16[0].shape), I16),
            "dlc": inp(f"{g.name}_dlc", list(g.dlc[0].shape), F16),
            "dlr": inp(f"{g.name}_dlr", list(g.dlr[0].shape), F16),
        }
    pidxu = inp("pred_idxu", list(pred.idxu[0].shape), I16)
    pidxi = inp("pred_idxi", list(pred.idxi[0].shape), I16)

    pred_out = nc.dram_tensor("pred_out", [P, pred.G_total], F32,
                              kind="ExternalOutput")
    import os
    kphase = os.environ.get("KPHASE", "full")
    dbg_spec = hp.get("dbg_spec")  # (name, rows, cols) of tensor to dump
    dbg_out = None
    if dbg_spec is not None:
        dbg_out = nc.dram_tensor("dbg_out", [dbg_spec[1], dbg_spec[2]], F32,
                                 kind="ExternalOutput")

    def internal(name, shape, shared=False, dt=F32):
        return nc.dram_tensor(name, list(shape), dt,
                              addr_space="Shared" if shared else "Local")

    u_tabs = [user_emb]
    it_tabs = [item_emb]
    u_shards = [u_shard0]
    it_shards = [it_shard0]
    fs_tab = {}      # (gat, l) -> full fs table
    fd_shard = {}    # (gat, l) -> local fd shard (fp16)
    ag_jobs = []     # (in_tensor, out_tensor)
    for l in range(L):
        for g, rows_in, rows_out in ((rate, US, UPAD), (rb, IS, IPAD), (tr, US, UPAD)):
            ai = internal(f"agin_fs_{g.name}{l}", [rows_in, D])
            ao = internal(f"fs_{g.name}{l}", [rows_out, D], shared=True)
            fs_tab[(g.name, l)] = (ai, ao)
        fd_shard[("rate", l)] = internal(f"fd_rate{l}", [IS, D], dt=F16)
        fd_shard[("rb", l)] = internal(f"fd_rb{l}", [US, D], dt=F16)
        fd_shard[("tr", l)] = internal(f"fd_tr{l}", [US, D], dt=F16)
        u_shards.append(internal(f"agin_u{l + 1}", [US, D]))
        u_tabs.append(internal(f"u{l + 1}", [UPAD, D], shared=True))
        it_shards.append(internal(f"agin_it{l + 1}", [IS, D]))
        it_tabs.append(internal(f"it{l + 1}", [IPAD, D], shared=True))
    q_sh = internal("q_sh", [US, D])
    p_sh = internal("p_sh", [US, D])
    hu_t = internal("hu", [UPAD, PD], dt=F16)
    hi_t = internal("hi", [IPAD, PD], dt=F16)

    rg = [list(range(N_CORES))]

    with tile.TileContext(nc) as tc, ExitStack() as topctx:
        const = topctx.enter_context(tc.tile_pool(name="const", bufs=1))

        def cload(t, shape, dt):
            s = const.tile(list(shape), dt, tag=f"c_{t.name}")
            nc.sync.dma_start(out=s[:], in_=t.ap()[:, :])
            return s

        im = cload(iota_m_in, [P, P], F16)
        ic = cload(iota_c_in, [P, 1], F16)
        onr = cload(ones_r_in, [1, P], F16)
        ident = const.tile([P, P], F32, tag='c_ident')
        make_identity(nc, ident[:])
        wu_sb = cload(wu, [D, L * 4 * D], F32)
        bu_sb = cload(bu, [P, L * 4 * D], F32)
        wi_sb = cload(wi, [D, L * 2 * D], F32)
        bi_sb = cload(bi_, [P, L * 2 * D], F32)
        a_sb = {nm: cload(a_in[nm], [P, L * D], F32) for nm in a_in}
        w1_sb = cload(w1, [2 * D, L * 2 * D], F32)
        b1_sb = cload(b1, [P, L * 2 * D], F32)
        w2_sb = cload(w2, [P, L * 2 * D], F32)
        b2_sb = cload(b2, [P, L * 2], F32)

        # ------------------------------------------------------------------
        def proj_phase(l):
            """Row-sharded projections + fd shards, then AllGather fs tables."""
            with ExitStack() as ctx:
                sb = ctx.enter_context(tc.tile_pool(name=f"proj{l}", bufs=2))
                ps = ctx.enter_context(
                    tc.tile_pool(name=f"projp{l}", bufs=4, space="PSUM"))
                pst = ctx.enter_context(
                    tc.tile_pool(name=f"projt{l}", bufs=2, space="PSUM"))

                BT = 8  # node tiles per iteration

                def do(shard_tab, n_tiles, w_sb_l, b_sb_l, ncols, outs):
                    # outs: list of (dst_tensor, col_lo, col_hi, fp16)
                    for t0 in range(0, n_tiles, BT):
                        bt = min(BT, n_tiles - t0)
                        src = sb.tile([P, BT * D], F32, tag="psrc")
                        nc.sync.dma_start(
                            out=src[:, :bt * D].rearrange("p (g d) -> p g d", d=D),
                            in_=shard_tab.ap()[t0 * P:(t0 + bt) * P, :]
                            .rearrange("(g p) d -> p g d", p=P))
                        big = sb.tile([P, BT * ncols], F32, tag="pbig")
                        for k in range(bt):
                            tp = pst.tile([D, P], F32, tag="ptp")
                            nc.tensor.transpose(
                                out=tp[:], in_=src[:, k * D:(k + 1) * D],
                                identity=ident[:])
                            uT = sb.tile([D, P], F32, tag="puT")
                            nc.vector.tensor_copy(out=uT[:], in_=tp[:])
                            mm = ps.tile([P, ncols], F32, tag="pmm")
                            nc.tensor.matmul(mm[:], lhsT=uT[:], rhs=w_sb_l,
                                             start=True, stop=True)
                            nc.vector.tensor_tensor(
                                out=big[:, k * ncols:(k + 1) * ncols],
                                in0=mm[:], in1=b_sb_l, op=Alu.add)
                        n16 = sum(1 for o in outs if o[3])
                        if n16:
                            w16 = sum(o[2] - o[1] for o in outs if o[3])
                            cv = sb.tile([P, BT * w16], F16, tag="pcv")
                            co = 0
                            for (dt_, lo, hi, f16) in outs:
                                if not f16:
                                    continue
                                wdt = hi - lo
                                nc.vector.tensor_copy(
                                    out=cv[:, :bt * w16]
                                    .rearrange("p (g d) -> p g d", d=w16)[:, :, co:co + wdt],
                                    in_=big[:, :bt * ncols]
                                    .rearrange("p (g d) -> p g d", d=ncols)[:, :, lo:hi])
                                co += wdt
                        co = 0
                        for (dt_, lo, hi, f16) in outs:
                            wdt = hi - lo
                            if f16:
                                nc.sync.dma_start(
                                    out=dt_.ap()[t0 * P:(t0 + bt) * P, :]
                                    .rearrange("(g p) d -> p g d", p=P),
                                    in_=cv[:, :bt * w16]
                                    .rearrange("p (g d) -> p g d", d=w16)[:, :, co:co + wdt])
                                co += wdt
                            else:
                                nc.sync.dma_start(
                                    out=dt_.ap()[t0 * P:(t0 + bt) * P, :]
                                    .rearrange("(g p) d -> p g d", p=P),
                                    in_=big[:, :bt * ncols]
                                    .rearrange("p (g d) -> p g d", d=ncols)[:, :, lo:hi])

                do(u_shards[l], UT,
                   wu_sb[:, l * 4 * D:(l + 1) * 4 * D], bu_sb[:, l * 4 * D:(l + 1) * 4 * D], 4 * D,
                   [(fs_tab[("rate", l)][0], 0, D, False),
                    (fs_tab[("tr", l)][0], D, 2 * D, False),
                    (fd_shard[("rb", l)], 2 * D, 3 * D, True),
                    (fd_shard[("tr", l)], 3 * D, 4 * D, True)])
                do(it_shards[l], IT,
                   wi_sb[:, l * 2 * D:(l + 1) * 2 * D], bi_sb[:, l * 2 * D:(l + 1) * 2 * D], 2 * D,
                   [(fd_shard[("rate", l)], 0, D, True),
                    (fs_tab[("rb", l)][0], D, 2 * D, False)])

            import os as _os3
            if _os3.environ.get("KNOAG") == "1":
                return
            for gname in ("rate", "rb", "tr"):
                ai, ao = fs_tab[(gname, l)]
                nc.gpsimd.collective_compute(
                    "AllGather", Alu.bypass, replica_groups=rg,
                    ins=[ai.ap()[:, :]], outs=[ao.ap()[:, :]])

        # ------------------------------------------------------------------
        def gats_phase(l, jobs):
            """Edge processing for all GATs of one layer, block-interleaved.

            jobs: list of (g, fs_table, fd_sh, out_tensor, resid_tab)."""
            import os as _os
            KG = int(_os.environ.get("KG", "9"))
            with ExitStack() as ctx:
                sb = ctx.enter_context(tc.tile_pool(name=f"egat{l}", bufs=2))
                ps_rep = ctx.enter_context(
                    tc.tile_pool(name=f"er{l}", bufs=3, space="PSUM"))
                ps_fde = ctx.enter_context(
                    tc.tile_pool(name=f"ef{l}", bufs=3, space="PSUM"))
                ps_acc = ctx.enter_context(
                    tc.tile_pool(name=f"ea{l}", bufs=2, space="PSUM"))

                def emit_block(job, wbi, w_base, g_base):
                    g, fs_table, fd_sh, out_tensor, resid_tab = job
                    gi = g_in[g.name]
                    a_l = a_sb[g.name][:, l * D:(l + 1) * D]
                    table_rows = fs_table.ap().shape[0]
                    K, Kb, nb = g.K, g.Kb, g.nb
                    G = wbi * K  # sub-tiles in block
                    # loads
                    idx_t = sb.tile([P, (g.WB * K * P) // 16], I16, tag="idx")
                    c0 = g_base * P // 16
                    nc.sync.dma_start(
                        out=idx_t[:, :G * P // 16],
                        in_=gi["idx"].ap()[:, c0:c0 + G * P // 16])
                    dlc_t = sb.tile([P, g.WB * K], F16, tag="dlc")
                    nc.sync.dma_start(out=dlc_t[:, :G],
                                      in_=gi["dlc"].ap()[:, g_base:g_base + G])
                    dlr_t = sb.tile([1, g.WB * K * P], F16, tag="dlr")
                    nc.sync.dma_start(
                        out=dlr_t[:, :G * P],
                        in_=gi["dlr"].ap()[:, g_base * P:(g_base + G) * P])
                    fd_t = sb.tile([P, g.WB * D], F16, tag="fd")
                    nc.sync.dma_start(
                        out=fd_t[:, :wbi * D],
                        in_=fd_sh.ap()[w_base * P:(w_base + wbi) * P, :]
                        .rearrange("(g p) d -> p g d", p=P))
                    if resid_tab is not None:
                        rs_t = sb.tile([P, g.WB * D], F32, tag="rs")
                        nc.sync.dma_start(
                            out=rs_t[:, :wbi * D],
                            in_=resid_tab.ap()[w_base * P:(w_base + wbi) * P, :]
                            .rearrange("(g p) d -> p g d", p=P))

                    fsg = sb.tile([P, g.WB * K * D], F32, tag="fsg")
                    # gathers per bank, round-robin over SWDGE queues
                    scol = 0
                    sg = 0
                    for b in (range(nb) if KG >= 1 else []):
                        ngb = wbi * Kb[b]          # sub-tiles for this bank
                        nidx = ngb * P
                        hi_row = min(table_rows, (b + 1) * BANK)
                        nc.gpsimd.dma_gather(
                            fsg[:, sg * D:(sg + ngb) * D]
                            .rearrange("p (g d) -> p g d", d=D),
                            fs_table.ap()[b * BANK:hi_row, :],
                            idx_t[:, scol:scol + nidx // 16],
                            nidx, nidx, D, single_packet=False,
                            queue_num=qctr[0] % 4)
                        qctr[0] += 1
                        scol += nidx // 16
                        sg += ngb
                    if KG <= 1:
                        out_t = sb.tile([P, g.WB * D], F32, tag="out")
                        nc.vector.tensor_copy(out=out_t[:, :wbi * D],
                                              in_=fsg[:, :wbi * D])
                        nc.sync.dma_start(
                            out=out_tensor.ap()[w_base * P:(w_base + wbi) * P, :]
                            .rearrange("(g p) d -> p g d", p=P),
                            in_=out_t[:, :wbi * D].rearrange("p (g d) -> p g d", d=D))
                        return
                    # acc psum for this block
                    acc = ps_acc.tile([P, g.WB * (D + 1)], F32, tag="acc",
                                      space="PSUM")

                    # sub-tile -> window map
                    win_of = []
                    for b in range(nb):
                        for wo in range(wbi):
                            win_of += [wo] * Kb[b]

                    # expansion one-hots for the whole block [P, G*P] fp16
                    # built per 4 sub-tiles (PSUM rep tile = 1 bank)
                    qt_big = sb.tile([P, g.WB * K * P], F16, tag="qtbig")
                    # scatter one-hots for the whole block: ONE DVE instr
                    if KG >= 4:
                        q_big = sb.tile([P, g.WB * K * P], F16, tag="qbig")
                        nc.vector.tensor_tensor(
                            out=q_big[:, :G * P]
                            .rearrange("p (g m) -> p g m", m=P),
                            in0=dlc_t[:, :G]
                            .rearrange("p (g m) -> p g m", m=1)
                            .to_broadcast([P, G, P]),
                            in1=im[:].rearrange("p (g m) -> p g m", g=1)
                            .to_broadcast([P, G, P]),
                            op=Alu.is_equal)

                    w_big = sb.tile([P, G * (D + 1)], F16, tag="wbig")
                    # X-stage groups of 8 sub-tiles
                    for x0 in range(0, G, 8):
                        xc = min(8, G - x0)
                        for r0 in range(x0, x0 + xc, 4):
                            rc = min(4, G - r0)
                            rep = ps_rep.tile([P, 4 * P], F32, tag="rep",
                                              space="PSUM")
                            nc.tensor.matmul(
                                rep[:, :rc * P], lhsT=onr[:],
                                rhs=dlr_t[:1, r0 * P:(r0 + rc) * P],
                                start=True, stop=True)
                            nc.vector.tensor_tensor(
                                out=qt_big[:, r0 * P:(r0 + rc) * P],
                                in0=ic[:].to_broadcast([P, rc * P]),
                                in1=rep[:, :rc * P],
                                op=Alu.is_equal)
                        fde = ps_fde.tile([P, 8 * D], F32, tag="fde", space="PSUM")
                        for j in range(xc):
                            gg = x0 + j
                            nc.tensor.matmul(
                                fde[:, j * D:(j + 1) * D],
                                lhsT=qt_big[:, gg * P:(gg + 1) * P],
                                rhs=fd_t[:, win_of[gg] * D:(win_of[gg] + 1) * D],
                                start=True, stop=True)
                        if KG == 2:
                            if x0 == 0:
                                out_t = sb.tile([P, g.WB * D], F32, tag="out")
                                nc.vector.tensor_copy(out=out_t[:, :wbi * D],
                                                      in_=fde[:, :wbi * D])
                            continue
                        # x = fs + fde ; leaky ; e ; z
                        x_t = sb.tile([P, 8 * D], F16, tag="x")
                        nc.vector.tensor_tensor(
                            out=x_t[:, :xc * D],
                            in0=fsg[:, x0 * D:(x0 + xc) * D],
                            in1=fde[:, :xc * D], op=Alu.add)
                        xs = sb.tile([P, 8 * D], F16, tag="xs")
                        nc.vector.tensor_scalar_mul(
                            xs[:, :xc * D], x_t[:, :xc * D], GAT_SLOPE)
                        xl = sb.tile([P, 8 * D], F16, tag="xl")
                        nc.vector.tensor_tensor(
                            out=xl[:, :xc * D], in0=x_t[:, :xc * D],
                            in1=xs[:, :xc * D], op=Alu.max)
                        xa = sb.tile([P, 8 * D], F32, tag="xa")
                        nc.vector.tensor_tensor(
                            out=xa[:, :xc * D], in0=xl[:, :xc * D],
                            in1=a_l.rearrange("p (g d) -> p g d", g=1)
                            .to_broadcast([P, xc, D]),
                            op=Alu.mult)
                        e8 = sb.tile([P, 8], F32, tag="e8")
                        nc.vector.reduce_sum(
                            out=e8[:, :xc],
                            in_=xa[:, :xc * D].rearrange("p (g d) -> p g d", d=D),
                            axis=mybir.AxisListType.X)
                        z8 = sb.tile([P, 8], F32, tag="z8")
                        nc.scalar.activation(z8[:, :xc], e8[:, :xc], Act.Exp)
                        nc.vector.tensor_tensor(
                            out=w_big[:, x0 * (D + 1):(x0 + xc) * (D + 1)]
                            .rearrange("p (g d) -> p g d", d=D + 1)[:, :, 0:D],
                            in0=fsg[:, x0 * D:(x0 + xc) * D]
                            .rearrange("p (g d) -> p g d", d=D),
                            in1=z8[:, :xc].rearrange("p (g d) -> p g d", d=1)
                            .to_broadcast([P, xc, D]),
                            op=Alu.mult)
                        nc.vector.tensor_copy(
                            out=w_big[:, x0 * (D + 1):(x0 + xc) * (D + 1)]
                            .rearrange("p (g d) -> p g d", d=D + 1)[:, :, D:D + 1],
                            in_=z8[:, :xc].rearrange("p (g d) -> p g d", d=1))
                        if KG == 3:
                            if x0 == 0:
                                out_t = sb.tile([P, g.WB * D], F32, tag="out")
                                nc.vector.tensor_copy(out=out_t[:, :wbi * D],
                                                      in_=w_big[:, :wbi * D])
                            continue

                    if KG >= 4:
                        # accumulation: window-major so PSUM groups in the
                        # shared bank are sequential, never interleaved
                        for wo in range(wbi):
                            subs = [gg for gg in range(G) if win_of[gg] == wo]
                            for si, gg in enumerate(subs):
                                nc.tensor.matmul(
                                    acc[:, wo * (D + 1):(wo + 1) * (D + 1)],
                                    lhsT=q_big[:, gg * P:(gg + 1) * P],
                                    rhs=w_big[:, gg * (D + 1):(gg + 1) * (D + 1)],
                                    start=(si == 0),
                                    stop=(si == len(subs) - 1))
                    if KG in (2, 3):
                        nc.sync.dma_start(
                            out=out_tensor.ap()[w_base * P:(w_base + wbi) * P, :]
                            .rearrange("(g p) d -> p g d", p=P),
                            in_=out_t[:, :wbi * D].rearrange("p (g d) -> p g d", d=D))
                        return
                    if KG == 4:
                        out_t = sb.tile([P, g.WB * D], F32, tag="out")
                        nc.vector.tensor_copy(
                            out=out_t[:, :wbi * D],
                            in_=acc[:, :g.WB * (D + 1)]
                            .rearrange("p (g d) -> p g d", d=D + 1)[:, :wbi, 0:D])
                        nc.sync.dma_start(
                            out=out_tensor.ap()[w_base * P:(w_base + wbi) * P, :]
                            .rearrange("(g p) d -> p g d", p=P),
                            in_=out_t[:, :wbi * D].rearrange("p (g d) -> p g d", d=D))
                        return
                    # divide + store (batched over the block's windows)
                    out_t = sb.tile([P, g.WB * D], F32, tag="out")
                    den = sb.tile([P, g.WB], F32, tag="den")
                    nc.vector.tensor_scalar_max(
                        den[:, :wbi].rearrange("p (w c) -> p w c", c=1),
                        acc[:, :wbi * (D + 1)]
                        .rearrange("p (w c) -> p w c", c=D + 1)[:, :, D:D + 1],
                        1e-30)
                    rec = sb.tile([P, g.WB], F32, tag="rec")
                    nc.vector.reciprocal(rec[:, :wbi], den[:, :wbi])
                    if resid_tab is None:
                        nc.vector.tensor_tensor(
                            out=out_t[:, :wbi * D]
                            .rearrange("p (w d) -> p w d", d=D),
                            in0=acc[:, :wbi * (D + 1)]
                            .rearrange("p (w c) -> p w c", c=D + 1)[:, :, 0:D],
                            in1=rec[:, :wbi]
                            .rearrange("p (w c) -> p w c", c=1)
                            .to_broadcast([P, wbi, D]),
                            op=Alu.mult)
                    else:
                        tmp = sb.tile([P, g.WB * D], F32, tag="dtmp")
                        nc.vector.tensor_tensor(
                            out=tmp[:, :wbi * D]
                            .rearrange("p (w d) -> p w d", d=D),
                            in0=acc[:, :wbi * (D + 1)]
                            .rearrange("p (w c) -> p w c", c=D + 1)[:, :, 0:D],
                            in1=rec[:, :wbi]
                            .rearrange("p (w c) -> p w c", c=1)
                            .to_broadcast([P, wbi, D]),
                            op=Alu.mult)
                        nc.vector.tensor_tensor(
                            out=out_t[:, :wbi * D], in0=tmp[:, :wbi * D],
                            in1=rs_t[:, :wbi * D], op=Alu.add)
                    nc.sync.dma_start(
                        out=out_tensor.ap()[w_base * P:(w_base + wbi) * P, :]
                        .rearrange("(g p) d -> p g d", p=P),
                        in_=out_t[:, :wbi * D].rearrange("p (g d) -> p g d", d=D))

                # interleave the jobs' blocks round-robin
                qctr = [0]
                cursors = []
                for job in jobs:
                    g = job[0]
                    w_base = 0
                    g_base = 0
                    blocks = []
                    for wbi in g.blocks:
                        blocks.append((wbi, w_base, g_base))
                        w_base += wbi
                        g_base += wbi * g.K
                    cursors.append(blocks)
                bi = 0
                while any(cursors):
                    for ji, job in enumerate(jobs):
                        if cursors[ji]:
                            emit_block(job, *cursors[ji].pop(0))
                    bi += 1

        # ------------------------------------------------------------------
        def epilogue_phase(l):
            with ExitStack() as ctx:
                sb = ctx.enter_context(tc.tile_pool(name=f"ep{l}", bufs=2))
                pst = ctx.enter_context(
                    tc.tile_pool(name=f"ept{l}", bufs=2, space="PSUM"))
                psm = ctx.enter_context(
                    tc.tile_pool(name=f"epm{l}", bufs=4, space="PSUM"))
                BT = 8
                w1_l = w1_sb[:, l * 2 * D:(l + 1) * 2 * D]
                b1_l = b1_sb[:, l * 2 * D:(l + 1) * 2 * D]
                w2_l = w2_sb[:, l * 2 * D:(l + 1) * 2 * D]
                b2_l = b2_sb[:, l * 2:(l + 1) * 2]
                for t0 in range(0, UT, BT):
                    bt = min(BT, UT - t0)
                    rows = slice(t0 * P, (t0 + bt) * P)
                    ut = sb.tile([P, BT * D], F32, tag="eu")
                    nc.sync.dma_start(
                        out=ut[:, :bt * D].rearrange("p (g d) -> p g d", d=D),
                        in_=u_shards[l].ap()[rows, :].rearrange("(g p) d -> p g d", p=P))
                    pt = sb.tile([P, BT * D], F32, tag="epp")
                    nc.sync.dma_start(
                        out=pt[:, :bt * D].rearrange("p (g d) -> p g d", d=D),
                        in_=p_sh.ap()[rows, :].rearrange("(g p) d -> p g d", p=P))
                    qt_ = sb.tile([P, BT * D], F32, tag="epq")
                    nc.sync.dma_start(
                        out=qt_[:, :bt * D].rearrange("p (g d) -> p g d", d=D),
                        in_=q_sh.ap()[rows, :].rearrange("(g p) d -> p g d", p=P))
                    ot = sb.tile([P, BT * D], F32, tag="eo")
                    for k in range(bt):
                        ts = []
                        for srcp in (ut, pt, qt_):
                            tp = pst.tile([D, P], F32, tag="etp")
                            nc.tensor.transpose(
                                out=tp[:], in_=srcp[:, k * D:(k + 1) * D],
                                identity=ident[:])
                            ts.append(tp)
                        ct_inf = sb.tile([2 * D, P], F32, tag="ecti")
                        nc.vector.tensor_copy(out=ct_inf[0:D, :], in_=ts[0][:])
                        nc.vector.tensor_copy(out=ct_inf[D:2 * D, :], in_=ts[1][:])
                        ct_int = sb.tile([2 * D, P], F32, tag="ectj")
                        nc.vector.tensor_copy(out=ct_int[0:D, :], in_=ts[0][:])
                        nc.vector.tensor_copy(out=ct_int[D:2 * D, :], in_=ts[2][:])
                        s2 = []
                        for ci, (ct, col) in enumerate(((ct_inf, 0), (ct_int, 1))):
                            mm = psm.tile([P, D], F32, tag="emm")
                            nc.tensor.matmul(
                                mm[:], lhsT=ct[:],
                                rhs=w1_l[:, col * D:(col + 1) * D],
                                start=True, stop=True)
                            s1 = sb.tile([P, D], F32, tag="es1")
                            nc.vector.tensor_tensor(
                                out=s1[:], in0=mm[:],
                                in1=b1_l[:, col * D:(col + 1) * D], op=Alu.add)
                            s1s = sb.tile([P, D], F32, tag="es1s")
                            nc.vector.tensor_scalar_mul(s1s[:], s1[:], MLP_SLOPE)
                            s1l = sb.tile([P, D], F32, tag="es1l")
                            nc.vector.tensor_tensor(
                                out=s1l[:], in0=s1[:], in1=s1s[:], op=Alu.max)
                            xw = sb.tile([P, D], F32, tag="exw")
                            nc.vector.tensor_tensor(
                                out=xw[:], in0=s1l[:],
                                in1=w2_l[:, col * D:(col + 1) * D], op=Alu.mult)
                            sv0 = sb.tile([P, 1], F32, tag="esv0")
                            nc.vector.reduce_sum(out=sv0[:], in_=xw[:],
                                                 axis=mybir.AxisListType.X)
                            sv = sb.tile([P, 1], F32, tag="esv")
                            nc.vector.tensor_tensor(
                                out=sv[:], in0=sv0[:],
                                in1=b2_l[:, col:col + 1], op=Alu.add)
                            svs = sb.tile([P, 1], F32, tag="esvs")
                            nc.vector.tensor_scalar_mul(svs[:], sv[:], MLP_SLOPE)
                            svl = sb.tile([P, 1], F32, tag="esvl")
                            nc.vector.tensor_tensor(
                                out=svl[:], in0=sv[:], in1=svs[:], op=Alu.max)
                            s2.append(svl)
                        dg = sb.tile([P, 1], F32, tag="edg")
                        nc.vector.tensor_tensor(
                            out=dg[:], in0=s2[0][:], in1=s2[1][:], op=Alu.subtract)
                        g0 = sb.tile([P, 1], F32, tag="eg0")
                        nc.scalar.activation(g0[:], dg[:], Act.Sigmoid)
                        pk = pt[:, k * D:(k + 1) * D]
                        qk = qt_[:, k * D:(k + 1) * D]
                        uk = ut[:, k * D:(k + 1) * D]
                        pq = sb.tile([P, D], F32, tag="epq2")
                        nc.vector.tensor_tensor(out=pq[:], in0=pk, in1=qk,
                                                op=Alu.subtract)
                        gpq = sb.tile([P, D], F32, tag="egpq")
                        nc.vector.tensor_scalar_mul(gpq[:], pq[:], g0[:, :1])
                        uq = sb.tile([P, D], F32, tag="euq")
                        nc.vector.tensor_tensor(out=uq[:], in0=uk, in1=qk,
                                                op=Alu.add)
                        nc.vector.tensor_tensor(
                            out=ot[:, k * D:(k + 1) * D], in0=uq[:], in1=gpq[:],
                            op=Alu.add)
                    nc.sync.dma_start(
                        out=u_shards[l + 1].ap()[rows, :]
                        .rearrange("(g p) d -> p g d", p=P),
                        in_=ot[:, :bt * D].rearrange("p (g d) -> p g d", d=D))

        # ------------------------------------------------------------------
        def hu_pred_phase():
            """Assemble hu/hi fp16 tables, then per-edge gathers + dots.

            One pool set so pred's hu-side gathers (ready once hu_t is
            written) overlap the hi_t build."""
            with ExitStack() as ctx:
                sb = ctx.enter_context(tc.tile_pool(name="hup", bufs=2))
                BT = 8
                for tabs, out_tab, n_tiles in ((u_tabs, hu_t, UPAD // P),
                                               (it_tabs, hi_t, IPAD // P)):
                    for t0 in range(0, n_tiles, BT):
                        bt = min(BT, n_tiles - t0)
                        rows = slice(t0 * P, (t0 + bt) * P)
                        big = sb.tile([P, BT * PD], F16, tag="hbig")
                        nc.vector.memset(big[:], 0)
                        for li, tab in enumerate(tabs):
                            ld = sb.tile([P, BT * D], F32, tag="hld")
                            nc.sync.dma_start(
                                out=ld[:, :bt * D].rearrange("p (g d) -> p g d", d=D),
                                in_=tab.ap()[rows, :]
                                .rearrange("(g p) d -> p g d", p=P))
                            nc.vector.tensor_copy(
                                out=big[:, :bt * PD]
                                .rearrange("p (g d) -> p g d", d=PD)
                                [:, :, li * D:(li + 1) * D],
                                in_=ld[:, :bt * D]
                                .rearrange("p (g d) -> p g d", d=D))
                        nc.sync.dma_start(
                            out=out_tab.ap()[rows, :]
                            .rearrange("(g p) d -> p g d", p=P),
                            in_=big[:, :bt * PD].rearrange("p (g d) -> p g d", d=PD))

                G = pred.G_blk
                for bi in range(pred.n_blocks):
                    hu_g = sb.tile([P, G * PD], F16, tag="phu")
                    hi_g = sb.tile([P, G * PD], F16, tag="phi")
                    iu_t = sb.tile([P, G * P // 16], I16, tag="piu")
                    c0 = bi * G * P // 16
                    nc.sync.dma_start(out=iu_t[:],
                                      in_=pidxu.ap()[:, c0:c0 + G * P // 16])
                    ii_t = sb.tile([P, G * P // 16], I16, tag="pii")
                    nc.sync.dma_start(out=ii_t[:],
                                      in_=pidxi.ap()[:, c0:c0 + G * P // 16])
                    # hu gathers: per user bank (spans its item-bank pairs)
                    sg = 0
                    scol = 0
                    qn = 0
                    for u_ in range(pred.nbu):
                        ngb = sum(pred.Kp[(u_, i_)] for i_ in range(pred.nbi))
                        nidx = ngb * P
                        hi_row = min(hu_t.ap().shape[0], (u_ + 1) * BANK)
                        nc.gpsimd.dma_gather(
                            hu_g[:, sg * PD:(sg + ngb) * PD]
                            .rearrange("p (g d) -> p g d", d=PD),
                            hu_t.ap()[u_ * BANK:hi_row, :],
                            iu_t[:, scol:scol + nidx // 16],
                            nidx, nidx, PD, single_packet=False,
                            queue_num=qn % 4)
                        qn += 1
                        sg += ngb
                        scol += nidx // 16
                    # hi gathers: per pair
                    sg = 0
                    scol = 0
                    for u_ in range(pred.nbu):
                        for i_ in range(pred.nbi):
                            ngb = pred.Kp[(u_, i_)]
                            nidx = ngb * P
                            hi_row = min(hi_t.ap().shape[0], (i_ + 1) * BANK)
                            nc.gpsimd.dma_gather(
                                hi_g[:, sg * PD:(sg + ngb) * PD]
                                .rearrange("p (g d) -> p g d", d=PD),
                                hi_t.ap()[i_ * BANK:hi_row, :],
                                ii_t[:, scol:scol + nidx // 16],
                                nidx, nidx, PD, single_packet=False,
                                queue_num=qn % 4)
                            qn += 1
                            sg += ngb
                            scol += nidx // 16
                    # dots (batched mult + per-group reduce)
                    dt_ = sb.tile([P, G], F32, tag="pdot")
                    for x0 in range(0, G, 8):
                        xc = min(8, G - x0)
                        prod = sb.tile([P, 8 * PD], F32, tag="pprod")
                        nc.vector.tensor_tensor(
                            out=prod[:, :xc * PD],
                            in0=hu_g[:, x0 * PD:(x0 + xc) * PD],
                            in1=hi_g[:, x0 * PD:(x0 + xc) * PD], op=Alu.mult)
                        nc.vector.reduce_sum(
                            out=dt_[:, x0:x0 + xc],
                            in_=prod[:, :xc * PD]
                            .rearrange("p (g d) -> p g d", d=PD),
                            axis=mybir.AxisListType.X)
                    nc.sync.dma_start(out=pred_out.ap()[:, bi * G:(bi + 1) * G],
                                      in_=dt_[:])

        # ------------------------------------------------------------------
        phase_order = []
        for l in range(L):
            phase_order += [f"proj{l}", f"gats{l}", f"agi{l}", f"epi{l}",
                            f"ag{l}"]
        phase_order += ["pred"]

        def run_until():
            if kphase == "null":
                return
            for ph in phase_order:
                l = int(ph[-1]) if ph[-1].isdigit() else 0
                if ph.startswith("proj"):
                    proj_phase(l)
                elif ph.startswith("gats"):
                    gats_phase(l, [
                        (rate, fs_tab[("rate", l)][1], fd_shard[("rate", l)],
                         it_shards[l + 1], it_shards[l]),
                        (rb, fs_tab[("rb", l)][1], fd_shard[("rb", l)],
                         q_sh, None),
                        (tr, fs_tab[("tr", l)][1], fd_shard[("tr", l)],
                         p_sh, None),
                    ])
                elif ph.startswith("epi"):
                    epilogue_phase(l)
                elif ph.startswith("agi"):
                    # item-table AllGather: input ready at end of gats, so
                    # this overlaps the epilogue on the collective engine
                    nc.gpsimd.collective_compute(
                        "AllGather", Alu.bypass, replica_groups=rg,
                        ins=[it_shards[l + 1].ap()[:, :]],
                        outs=[it_tabs[l + 1].ap()[:, :]])
                elif ph.startswith("ag"):
                    nc.gpsimd.collective_compute(
                        "AllGather", Alu.bypass, replica_groups=rg,
                        ins=[u_shards[l + 1].ap()[:, :]],
                        outs=[u_tabs[l + 1].ap()[:, :]])
                elif ph == "pred":
                    hu_pred_phase()
                if ph == kphase:
                    return

        run_until()
        if dbg_out is not None:
            dbg_tensors = dict(
                q_sh=q_sh, p_sh=p_sh, hu=hu_t, hi=hi_t,
                **{f"u_shard{i}": t for i, t in enumerate(u_shards)},
                **{f"it_shard{i}": t for i, t in enumerate(it_shards)},
                **{f"u_tab{i}": t for i, t in enumerate(u_tabs)},
                **{f"it_tab{i}": t for i, t in enumerate(it_tabs)},
                **{f"fs_{nm}{l}": fs_tab[(nm, l)][1] for nm in ("rate", "rb", "tr")
                   for l in range(L)},
                **{f"fsin_{nm}{l}": fs_tab[(nm, l)][0] for nm in ("rate", "rb", "tr")
                   for l in range(L)},
                **{f"fd_{nm}{l}": fd_shard[(nm, l)] for nm in ("rate", "rb", "tr")
                   for l in range(L)},
            )
            src_t = dbg_tensors[dbg_spec[0]]
            sdt = src_t.ap().dtype
            with ExitStack() as ctx:
                sbd = ctx.enter_context(tc.tile_pool(name="dbg", bufs=2))
                rows, cols = dbg_spec[1], dbg_spec[2]
                for r0 in range(0, rows, P):
                    rc = min(P, rows - r0)
                    t_ = sbd.tile([P, cols], sdt, tag="dbg")
                    nc.sync.dma_start(out=t_[:rc, :],
                                      in_=src_t.ap()[r0:r0 + rc, :])
                    if sdt != F32:
                        t2 = sbd.tile([P, cols], F32, tag="dbg2")
                        nc.vector.tensor_copy(out=t2[:rc, :], in_=t_[:rc, :])
                        t_ = t2
                    nc.sync.dma_start(out=dbg_out.ap()[r0:r0 + rc, :],
                                      in_=t_[:rc, :])

    nc.compile()
    return nc


# ---------------------------------------------------------------------------
# entry point
# ---------------------------------------------------------------------------

def _pad_rows(a, rows):
    out = np.zeros((rows, a.shape[1]), dtype=a.dtype)
    out[:a.shape[0]] = a
    return out


def kernel(**inputs):
    U, D = inputs["user_emb"].shape
    I = inputs["item_emb"].shape[0]
    L = inputs["rate_Ws"].shape[0]
    UT = _ceil(_ceil(U, P), N_CORES)
    IT = _ceil(_ceil(I, P), N_CORES)
    US, IS = UT * P, IT * P
    UPAD, IPAD = US * N_CORES, IS * N_CORES
    PD = _ceil(D * (L + 1), P) * P if D * (L + 1) % P else D * (L + 1)
    # gather elem size must be a multiple of 256 bytes -> PD*2 % 256 == 0
    PD = _ceil(D * (L + 1) * 2, 256) * 128

    rate_src = np.asarray(inputs["rate_src"])
    rate_dst = np.asarray(inputs["rate_dst"])
    trust_src = np.asarray(inputs["trust_src"])
    trust_dst = np.asarray(inputs["trust_dst"])

    rate = GatStruct("rate", rate_src, rate_dst, UPAD, IT)
    rb = GatStruct("rb", rate_dst, rate_src, IPAD, UT)
    tr = GatStruct("tr", trust_src, trust_dst, UPAD, UT)

    pos_src = np.asarray(inputs["pos_src"])
    pos_dst = np.asarray(inputs["pos_dst"])
    neg_src = np.asarray(inputs["neg_src"])
    neg_dst = np.asarray(inputs["neg_dst"])
    psrc = np.concatenate([pos_src, neg_src])
    pdst = np.concatenate([pos_dst, neg_dst])
    pred = PredStruct(psrc, pdst, UPAD, IPAD, block_edges=9216)

    import os
    hp = dict(U=U, I=I, D=D, L=L, UT=UT, IT=IT, PD=PD,
              rate=rate, rb=rb, tr=tr, pred=pred)
    print(f"[kernel] struct: rate K={rate.K} Kb={rate.Kb} WB={rate.WB} blocks={len(rate.blocks)}; "
          f"rb K={rb.K} WB={rb.WB} blocks={len(rb.blocks)}; "
          f"tr K={tr.K} WB={tr.WB} blocks={len(tr.blocks)}; "
          f"pred G_blk={pred.G_blk} blocks={pred.n_blocks}")
    kdbg = os.environ.get("KDBG")
    if kdbg:
        shp = {}
        for i in range(L + 1):
            shp[f"u_shard{i}"] = (US, D); shp[f"it_shard{i}"] = (IS, D)
            shp[f"u_tab{i}"] = (UPAD, D); shp[f"it_tab{i}"] = (IPAD, D)
        for l in range(L):
            shp[f"fs_rate{l}"] = (UPAD, D); shp[f"fs_tr{l}"] = (UPAD, D)
            shp[f"fs_rb{l}"] = (IPAD, D)
            shp[f"fsin_rate{l}"] = (US, D); shp[f"fsin_tr{l}"] = (US, D)
            shp[f"fsin_rb{l}"] = (IS, D)
            shp[f"fd_rate{l}"] = (IS, D); shp[f"fd_rb{l}"] = (US, D)
            shp[f"fd_tr{l}"] = (US, D)
        shp["q_sh"] = (US, D); shp["p_sh"] = (US, D)
        shp["hu"] = (UPAD, PD); shp["hi"] = (IPAD, PD)
        hp["dbg_spec"] = (kdbg, *shp[kdbg])

    t_b = __import__("time").time()
    nc = build_program(hp)
    print(f"[kernel] build+compile: {__import__('time').time() - t_b:.1f}s")

    # ---- inputs ----
    f16 = NPF16
    ue_pad = _pad_rows(inputs["user_emb"].astype(np.float32), UPAD)
    ie_pad = _pad_rows(inputs["item_emb"].astype(np.float32), IPAD)
    wu = np.concatenate([
        np.concatenate([inputs["rate_Ws"][l], inputs["tr_Ws"][l],
                        inputs["rb_Wd"][l], inputs["tr_Wd"][l]], axis=1)
        for l in range(L)], axis=1).astype(np.float32)
    bu = np.concatenate([
        np.tile(np.concatenate([inputs["rate_bs"][l], inputs["tr_bs"][l],
                                inputs["rb_bd"][l], inputs["tr_bd"][l]])[None, :],
                (P, 1))
        for l in range(L)], axis=1).astype(np.float32)
    wi = np.concatenate([
        np.concatenate([inputs["rate_Wd"][l], inputs["rb_Ws"][l]], axis=1)
        for l in range(L)], axis=1).astype(np.float32)
    bi_ = np.concatenate([
        np.tile(np.concatenate([inputs["rate_bd"][l], inputs["rb_bs"][l]])[None, :],
                (P, 1))
        for l in range(L)], axis=1).astype(np.float32)
    a_arrs = {}
    for nm in ("rate", "rb", "tr"):
        a_arrs[nm] = np.concatenate([
            np.tile(np.asarray(inputs[f"{nm}_a"][l])[None, :], (P, 1))
            for l in range(L)], axis=1).astype(np.float32)
    w1 = np.concatenate([
        np.concatenate([inputs["inf_W1"][l], inputs["int_W1"][l]], axis=1)
        for l in range(L)], axis=1).astype(np.float32)
    b1 = np.concatenate([
        np.tile(np.concatenate([inputs["inf_b1"][l], inputs["int_b1"][l]])[None, :],
                (P, 1))
        for l in range(L)], axis=1).astype(np.float32)
    w2 = np.concatenate([
        np.tile(np.concatenate([inputs["inf_W2"][l][:, 0],
                                inputs["int_W2"][l][:, 0]])[None, :], (P, 1))
        for l in range(L)], axis=1).astype(np.float32)
    b2 = np.concatenate([
        np.tile(np.array([[inputs["inf_b2"][l][0], inputs["int_b2"][l][0]]],
                         dtype=np.float32), (P, 1))
        for l in range(L)], axis=1).astype(np.float32)
    iota = np.arange(P, dtype=np.float32)
    iota_m = np.tile(iota[None, :], (P, 1)).astype(f16)
    iota_c = iota[:, None].astype(f16)
    ones_r = np.ones((1, P), dtype=f16)

    in_maps = []
    for c in range(N_CORES):
        m = {
            "user_emb": ue_pad, "item_emb": ie_pad,
            "u_shard0": ue_pad[c * US:(c + 1) * US],
            "it_shard0": ie_pad[c * IS:(c + 1) * IS],
            "wu": wu, "bu": bu, "wi": wi, "bi": bi_,
            "a_rate": a_arrs["rate"], "a_rb": a_arrs["rb"], "a_tr": a_arrs["tr"],
            "w1": w1, "b1": b1, "w2": w2, "b2": b2,
            "iota_m": iota_m, "iota_c": iota_c, "ones_r": ones_r,
            "pred_idxu": pred.idxu[c], "pred_idxi": pred.idxi[c],
        }
        for g in (rate, rb, tr):
            m[f"{g.name}_idx"] = g.idx16[c]
            m[f"{g.name}_dlc"] = g.dlc[c]
            m[f"{g.name}_dlr"] = g.dlr[c]
        in_maps.append(m)

    trace = os.environ.get("KTRACE") == "1"
    global LAST_RES, LAST_HP, LAST_EXEC_NS
    if os.environ.get("KSKIPRUN") == "1":
        class _FakeRes:
            results = [{"pred_out": np.zeros((P, pred.G_total), np.float32)}
                       for _ in range(N_CORES)]
            exec_time_ns = None
        res = _FakeRes()
    else:
        t_run = __import__("time").time()
        res = run_bass_kernel_spmd(nc, in_maps, core_ids=list(range(N_CORES)),
                                   trace=trace)
        print(f"[kernel] device run wall: {__import__('time').time() - t_run:.1f}s")
    LAST_RES, LAST_HP, LAST_EXEC_NS = res, hp, res.exec_time_ns
    if os.environ.get("KBENCH") == "1":
        tmin = bench_pjrt(nc, in_maps, iters=int(os.environ.get("KBENCH_ITERS", "4")))
        LAST_EXEC_NS = int(tmin * 1e9)

    # ---- assemble outputs ----
    E = len(psrc)
    out = np.zeros((E,), dtype=np.float32)
    for c in range(N_CORES):
        vals = res.results[c]["pred_out"]  # [128, G_total]
        smap = pred.slotmap[c]
        flat = vals.T.reshape(-1)          # slot s = (p, g) -> g*128 + p? no:
        # slot i = (partition i%128, group i//128) => value at vals[i%128, i//128]
        gidx = np.arange(len(smap))
        v = vals[gidx % P, gidx // P]
        ok = smap >= 0
        out[smap[ok]] = v[ok]
    pos = out[:len(pos_src)].reshape(-1, 1)
    neg = out[len(pos_src):].reshape(-1, 1)
    return pos, neg



# revision 43
# speedup vs baseline: 1.0056x; 1.0056x over previous
"""DiffNet++ (GATv2 diffusion + gamma gating + dot-product prediction) on 8
Trainium2 NeuronCores via Bass/Tile.

Strategy (dst-range edge sharding, one SPMD program):
  - Users/items row-sharded equally: users 98 tiles (12544 rows)/core, items 49
    tiles (6272 rows)/core. Each GAT edge belongs to the core owning its dst.
  - Per core, edges are grouped by dst "window" (128 rows) and bucket-sorted by
    src-table bank (dma_gather int16 index => 32768-row banks). The padded slot
    structure is the max over cores, so one program serves all cores.
  - Segment softmax without max subtraction (logits ~1e-2): out[v] =
    (sum_e exp(e) fs[src]) / (sum_e exp(e)), accumulated via one-hot matmuls
    into PSUM windows; per-dst divide afterwards.
  - fs[src] rows: dma_gather (batched indirect DMA) spread round-robin over 4
    SWDGE queues (gathers are descriptor-rate-bound; one queue caps in-flight
    descriptors). fd[dst]: expanded from the contiguous dst window by one-hot
    fp16 matmuls; one-hots built batched (1 DVE is_equal per 4 sub-tiles for
    expansion, 1 per block for scatter); scatter matmuls in fp16.
  - The 3 GATs of a layer are emitted block-interleaved in one phase/pool set
    so gather DMA streams overlap the other GATs' DVE/PE compute.
  - Projections row-sharded + AllGather (Shared outputs = fast collective
    path); updated embeddings AllGather/layer (Shared).
  - Prediction: hu/hi concat tables in fp16 padded to 256 cols; gather both
    sides per edge; fused multiply-reduce dots.
"""
import sys

sys.path.insert(0, "/opt/trn_rl_repo")

from contextlib import ExitStack

import numpy as np
import ml_dtypes

import concourse.bass as bass
import concourse.tile as tile
from concourse import bacc, mybir
from concourse.bass_utils import run_bass_kernel_spmd
from concourse.masks import make_identity

N_CORES = 8
P = 128
BANK = 32768
GAT_SLOPE = 0.2
MLP_SLOPE = 0.01
F16 = mybir.dt.float16
F32 = mybir.dt.float32
I16 = mybir.dt.int16
NPF16 = np.dtype("float16")

Alu = mybir.AluOpType
Act = mybir.ActivationFunctionType


def _ceil(a, b):
    return -(-a // b)


# ---------------------------------------------------------------------------
# host-side preprocessing
# ---------------------------------------------------------------------------

class GatStruct:
    """Canonical (core-uniform) structure for one GAT graph's edges."""

    def __init__(self, name, src, dst, table_rows, shard_tiles):
        self.name = name
        self.nb = _ceil(table_rows, BANK)
        self.shard_tiles = shard_tiles
        S = shard_tiles * P
        self.S = S

        core = np.minimum(dst // S, N_CORES - 1)
        win = (dst - core * S) // P
        bank = src // BANK

        cnt = np.zeros((N_CORES, shard_tiles, self.nb), dtype=np.int64)
        np.add.at(cnt, (core, win, bank), 1)
        self.Kb = [max(1, int(_ceil(int(cnt[:, :, b].max()), P)))
                   for b in range(self.nb)]
        self.K = sum(self.Kb)
        self.WB = max(1, min(7, 80 // self.K))
        self.blocks = []
        t = shard_tiles
        while t > 0:
            wbi = min(self.WB, t)
            self.blocks.append(wbi)
            t -= wbi
        self.G_total = shard_tiles * self.K  # sub-tiles per core overall
        self.total_cols = self.G_total * P // 16

        order = np.lexsort((bank, win, core))
        src_s = src[order]
        dst_s = dst[order]
        core_s = core[order]
        win_s = win[order]
        bank_s = bank[order]

        self.idx16 = []
        self.dlc = []
        self.dlr = []
        for c in range(N_CORES):
            sel = core_s == c
            csrc = src_s[sel]
            cdst = dst_s[sel]
            cwin = win_s[sel]
            cbank = bank_s[sel]
            key = cwin.astype(np.int64) * self.nb + cbank
            ids = np.zeros((self.G_total * P,), dtype=np.int16)
            dl = np.full((self.G_total * P,), -1.0, dtype=NPF16)
            # slot layout: per block: [bank b: [window wo: Kb[b]*128 slots]]
            slot0 = 0
            w_base = 0
            for wbi in self.blocks:
                for b in range(self.nb):
                    for wo in range(wbi):
                        w = w_base + wo
                        e0 = np.searchsorted(key, w * self.nb + b, "left")
                        e1 = np.searchsorted(key, w * self.nb + b, "right")
                        n = e1 - e0
                        nsw = self.Kb[b] * P
                        assert n <= nsw, (name, c, w, b, n, nsw)
                        ids[slot0:slot0 + n] = (csrc[e0:e1] - b * BANK).astype(np.int16)
                        dl[slot0:slot0 + n] = (cdst[e0:e1] - (c * S + w * P)).astype(NPF16)
                        slot0 += nsw
                w_base += wbi
            assert slot0 == self.G_total * P
            cols = self.total_cols
            a = np.empty((16, cols), dtype=np.int16)
            j = np.arange(self.G_total * P)
            a[j % 16, j // 16] = ids
            self.idx16.append(np.tile(a, (8, 1)))
            self.dlc.append(np.ascontiguousarray(
                dl.reshape(self.G_total, P).T))          # [128, G_total]
            self.dlr.append(dl.reshape(1, -1).copy())    # [1, G_total*128]


class PredStruct:
    """Canonical structure for prediction edges (pos+neg concatenated)."""

    def __init__(self, src, dst, u_rows, i_rows, block_edges):
        E = len(src)
        assert E % N_CORES == 0
        per_core = E // N_CORES
        self.per_core = per_core
        self.nbu = _ceil(u_rows, BANK)
        self.nbi = _ceil(i_rows, BANK)
        self.n_blocks = _ceil(per_core, block_edges)
        pairs = [(u_, i_) for u_ in range(self.nbu) for i_ in range(self.nbi)]
        self.pairs = pairs

        core = np.arange(E) // per_core
        blk = (np.arange(E) % per_core) // block_edges
        ub = src // BANK
        ib = dst // BANK
        cnt = np.zeros((N_CORES, self.n_blocks, self.nbu, self.nbi), dtype=np.int64)
        np.add.at(cnt, (core, blk, ub, ib), 1)
        self.Kp = {pq: max(1, int(_ceil(int(cnt[:, :, pq[0], pq[1]].max()), P)))
                   for pq in pairs}
        self.G_blk = sum(self.Kp[pq] for pq in pairs)
        self.G_total = self.G_blk * self.n_blocks

        self.idxu = []
        self.idxi = []
        self.slotmap = []
        for c in range(N_CORES):
            lo = c * per_core
            cs = src[lo:lo + per_core]
            cd = dst[lo:lo + per_core]
            iu = np.zeros((self.G_total * P,), dtype=np.int16)
            ii = np.zeros((self.G_total * P,), dtype=np.int16)
            smap = np.full((self.G_total * P,), -1, dtype=np.int64)
            for bi in range(self.n_blocks):
                b0 = bi * block_edges
                b1 = min(b0 + block_edges, per_core)
                bs, bd = cs[b0:b1], cd[b0:b1]
                bub, bib = bs // BANK, bd // BANK
                key = bub.astype(np.int64) * self.nbi + bib
                ordk = np.argsort(key, kind="stable")
                keys = key[ordk]
                off = bi * self.G_blk * P
                for pq in pairs:
                    kv = pq[0] * self.nbi + pq[1]
                    e0 = np.searchsorted(keys, kv, "left")
                    e1 = np.searchsorted(keys, kv, "right")
                    n = e1 - e0
                    npad = self.Kp[pq] * P
                    assert n <= npad
                    sel2 = ordk[e0:e1]
                    iu[off:off + n] = (bs[sel2] - pq[0] * BANK).astype(np.int16)
                    ii[off:off + n] = (bd[sel2] - pq[1] * BANK).astype(np.int16)
                    smap[off:off + n] = lo + b0 + sel2
                    off += npad
            cols = self.G_total * P // 16
            j = np.arange(self.G_total * P)
            au = np.empty((16, cols), dtype=np.int16)
            au[j % 16, j // 16] = iu
            ai = np.empty((16, cols), dtype=np.int16)
            ai[j % 16, j // 16] = ii
            self.idxu.append(np.tile(au, (8, 1)))
            self.idxi.append(np.tile(ai, (8, 1)))
            self.slotmap.append(smap)


# ---------------------------------------------------------------------------
# program builder
# ---------------------------------------------------------------------------

def bench_pjrt(nc, in_maps, iters=3):
    """Time steady-state executions of the compiled program on the 8 cores.

    Rebuilds the PJRT callable without donation, uploads inputs once, then
    times back-to-back executions."""
    import time as _time
    import jax
    from jax.sharding import Mesh, PartitionSpec
    from jax.experimental.shard_map import shard_map
    from concourse import bass2jax
    from concourse import mybir as _mb

    bass2jax.install_neuronx_cc_hook()
    partition_name = (nc.partition_id_tensor.name
                      if nc.partition_id_tensor else None)
    in_names, out_names, out_avals = [], [], []
    for alloc in nc.m.functions[0].allocations:
        if not isinstance(alloc, _mb.MemoryLocationSet):
            continue
        name = alloc.memorylocations[0].name
        if alloc.kind == "ExternalInput":
            if name != partition_name:
                in_names.append(name)
        elif alloc.kind == "ExternalOutput":
            out_names.append(name)
            out_avals.append(jax.core.ShapedArray(
                tuple(alloc.tensor_shape), _mb.dt.np(alloc.dtype)))
    n_params = len(in_names)
    zero_outs = [np.zeros(a.shape, a.dtype) for a in out_avals]
    all_names = in_names + out_names
    if partition_name is not None:
        all_names = all_names + [partition_name]

    def _body(*args):
        operands = list(args)
        if partition_name is not None:
            operands.append(bass2jax.partition_id_tensor())
        return tuple(bass2jax._bass_exec_p.bind(
            *operands, out_avals=tuple(out_avals),
            in_names=tuple(all_names), out_names=tuple(out_names),
            lowering_input_output_aliases=(), sim_require_finite=True,
            sim_require_nnan=True, nc=nc))

    devices = jax.devices()[:N_CORES]
    mesh = Mesh(np.asarray(devices), ("core",))
    nspec = n_params + len(out_names)
    f = jax.jit(shard_map(_body, mesh=mesh,
                          in_specs=(PartitionSpec("core"),) * nspec,
                          out_specs=(PartitionSpec("core"),) * len(out_names),
                          check_rep=False), keep_unused=True)
    from jax.sharding import NamedSharding
    sh = NamedSharding(mesh, PartitionSpec("core"))
    concat_in = [np.concatenate([np.asarray(m[nm]) for m in in_maps], axis=0)
                 for nm in in_names]
    concat_in += [np.concatenate([z] * N_CORES, axis=0) for z in zero_outs]
    dev_in = [jax.device_put(x, sh) for x in concat_in]
    times = []
    for i in range(iters):
        t0 = _time.time()
        outs = f(*dev_in)
        jax.block_until_ready(outs)
        times.append(_time.time() - t0)
    print(f"[bench] iter times: {[f'{t*1e3:.2f}ms' for t in times]}")
    # pipelined: issue PIPE calls back-to-back, block once
    PIPE = int(os.environ.get("KPIPE", "128")) if (os := __import__("os")) else 128
    outs = [f(*dev_in) for _ in range(2)]
    jax.block_until_ready(outs)  # warm
    t0 = _time.time()
    outs = [f(*dev_in) for _ in range(PIPE)]
    jax.block_until_ready(outs)
    piped = (_time.time() - t0) / PIPE
    print(f"[bench] pipelined per-iter: {piped*1e3:.2f}ms")
    return min(min(times[1:]) if len(times) > 1 else times[0], piped)


def build_program(hp):
    U, I, D, L = hp["U"], hp["I"], hp["D"], hp["L"]
    UT, IT = hp["UT"], hp["IT"]
    US, IS = UT * P, IT * P
    UPAD, IPAD = US * N_CORES, IS * N_CORES
    rate, rb, tr = hp["rate"], hp["rb"], hp["tr"]
    pred = hp["pred"]
    PD = hp["PD"]
    CD = D * (L + 1)

    nc = bacc.Bacc("TRN2", target_bir_lowering=False, debug=False,
                   num_devices=N_CORES, num_swdge_queues=4)

    def inp(name, shape, dt):
        return nc.dram_tensor(name, list(shape), dt, kind="ExternalInput")

    user_emb = inp("user_emb", [UPAD, D], F32)       # full, padded
    item_emb = inp("item_emb", [IPAD, D], F32)
    u_shard0 = inp("u_shard0", [US, D], F32)         # per-core slice
    it_shard0 = inp("it_shard0", [IS, D], F32)
    wu = inp("wu", [D, L * 4 * D], F32)
    bu = inp("bu", [P, L * 4 * D], F32)
    wi = inp("wi", [D, L * 2 * D], F32)
    bi_ = inp("bi", [P, L * 2 * D], F32)
    a_in = {g.name: inp(f"a_{g.name}", [P, L * D], F32) for g in (rate, rb, tr)}
    w1 = inp("w1", [2 * D, L * 2 * D], F32)
    b1 = inp("b1", [P, L * 2 * D], F32)
    w2 = inp("w2", [P, L * 2 * D], F32)
    b2 = inp("b2", [P, L * 2], F32)
    iota_m_in = inp("iota_m", [P, P], F16)
    iota_c_in = inp("iota_c", [P, 1], F16)
    ones_r_in = inp("ones_r", [1, P], F16)

    g_in = {}
    for g in (rate, rb, tr):
        g_in[g.name] = {
            "idx": inp(f"{g.name}_idx", list(g.idx16[0].shape), I16),
            "dlc": inp(f"{g.name}_dlc", list(g.dlc[0].shape), F16),
            "dlr": inp(f"{g.name}_dlr", list(g.dlr[0].shape), F16),
        }
    pidxu = inp("pred_idxu", list(pred.idxu[0].shape), I16)
    pidxi = inp("pred_idxi", list(pred.idxi[0].shape), I16)

    pred_out = nc.dram_tensor("pred_out", [P, pred.G_total], F32,
                              kind="ExternalOutput")
    import os
    kphase = os.environ.get("KPHASE", "full")
    dbg_spec = hp.get("dbg_spec")  # (name, rows, cols) of tensor to dump
    dbg_out = None
    if dbg_spec is not None:
        dbg_out = nc.dram_tensor("dbg_out", [dbg_spec[1], dbg_spec[2]], F32,
                                 kind="ExternalOutput")

    def internal(name, shape, shared=False, dt=F32):
        return nc.dram_tensor(name, list(shape), dt,
                              addr_space="Shared" if shared else "Local")

    u_tabs = [user_emb]
    it_tabs = [item_emb]
    u_shards = [u_shard0]
    it_shards = [it_shard0]
    fs_tab = {}      # (gat, l) -> full fs table
    fd_shard = {}    # (gat, l) -> local fd shard (fp16)
    ag_jobs = []     # (in_tensor, out_tensor)
    for l in range(L):
        for g, rows_in, rows_out in ((rate, US, UPAD), (rb, IS, IPAD), (tr, US, UPAD)):
            ai = internal(f"agin_fs_{g.name}{l}", [rows_in, D])
            ao = internal(f"fs_{g.name}{l}", [rows_out, D], shared=True)
            fs_tab[(g.name, l)] = (ai, ao)
        fd_shard[("rate", l)] = internal(f"fd_rate{l}", [IS, D], dt=F16)
        fd_shard[("rb", l)] = internal(f"fd_rb{l}", [US, D], dt=F16)
        fd_shard[("tr", l)] = internal(f"fd_tr{l}", [US, D], dt=F16)
        u_shards.append(internal(f"agin_u{l + 1}", [US, D]))
        u_tabs.append(internal(f"u{l + 1}", [UPAD, D], shared=True))
        it_shards.append(internal(f"agin_it{l + 1}", [IS, D]))
        it_tabs.append(internal(f"it{l + 1}", [IPAD, D], shared=True))
    q_sh = internal("q_sh", [US, D])
    p_sh = internal("p_sh", [US, D])
    hu_t = internal("hu", [UPAD, PD], dt=F16)
    hi_t = internal("hi", [IPAD, PD], dt=F16)

    rg = [list(range(N_CORES))]

    with tile.TileContext(nc) as tc, ExitStack() as topctx:
        const = topctx.enter_context(tc.tile_pool(name="const", bufs=1))

        def cload(t, shape, dt):
            s = const.tile(list(shape), dt, tag=f"c_{t.name}")
            nc.sync.dma_start(out=s[:], in_=t.ap()[:, :])
            return s

        im = cload(iota_m_in, [P, P], F16)
        ic = cload(iota_c_in, [P, 1], F16)
        onr = cload(ones_r_in, [1, P], F16)
        ident = const.tile([P, P], F32, tag='c_ident')
        make_identity(nc, ident[:])
        wu_sb = cload(wu, [D, L * 4 * D], F32)
        bu_sb = cload(bu, [P, L * 4 * D], F32)
        wi_sb = cload(wi, [D, L * 2 * D], F32)
        bi_sb = cload(bi_, [P, L * 2 * D], F32)
        a_sb = {nm: cload(a_in[nm], [P, L * D], F32) for nm in a_in}
        w1_sb = cload(w1, [2 * D, L * 2 * D], F32)
        b1_sb = cload(b1, [P, L * 2 * D], F32)
        w2_sb = cload(w2, [P, L * 2 * D], F32)
        b2_sb = cload(b2, [P, L * 2], F32)

        # ------------------------------------------------------------------
        def proj_phase(l):
            """Row-sharded projections + fd shards, then AllGather fs tables."""
            with ExitStack() as ctx:
                sb = ctx.enter_context(tc.tile_pool(name=f"proj{l}", bufs=2))
                ps = ctx.enter_context(
                    tc.tile_pool(name=f"projp{l}", bufs=4, space="PSUM"))
                pst = ctx.enter_context(
                    tc.tile_pool(name=f"projt{l}", bufs=2, space="PSUM"))

                BT = 8  # node tiles per iteration

                def do(shard_tab, n_tiles, w_sb_l, b_sb_l, ncols, outs):
                    # outs: list of (dst_tensor, col_lo, col_hi, fp16)
                    for t0 in range(0, n_tiles, BT):
                        bt = min(BT, n_tiles - t0)
                        src = sb.tile([P, BT * D], F32, tag="psrc")
                        nc.sync.dma_start(
                            out=src[:, :bt * D].rearrange("p (g d) -> p g d", d=D),
                            in_=shard_tab.ap()[t0 * P:(t0 + bt) * P, :]
                            .rearrange("(g p) d -> p g d", p=P))
                        big = sb.tile([P, BT * ncols], F32, tag="pbig")
                        for k in range(bt):
                            tp = pst.tile([D, P], F32, tag="ptp")
                            nc.tensor.transpose(
                                out=tp[:], in_=src[:, k * D:(k + 1) * D],
                                identity=ident[:])
                            uT = sb.tile([D, P], F32, tag="puT")
                            nc.vector.tensor_copy(out=uT[:], in_=tp[:])
                            mm = ps.tile([P, ncols], F32, tag="pmm")
                            nc.tensor.matmul(mm[:], lhsT=uT[:], rhs=w_sb_l,
                                             start=True, stop=True)
                            nc.vector.tensor_tensor(
                                out=big[:, k * ncols:(k + 1) * ncols],
                                in0=mm[:], in1=b_sb_l, op=Alu.add)
                        n16 = sum(1 for o in outs if o[3])
                        if n16:
                            w16 = sum(o[2] - o[1] for o in outs if o[3])
                            cv = sb.tile([P, BT * w16], F16, tag="pcv")
                            co = 0
                            for (dt_, lo, hi, f16) in outs:
                                if not f16:
                                    continue
                                wdt = hi - lo
                                nc.vector.tensor_copy(
                                    out=cv[:, :bt * w16]
                                    .rearrange("p (g d) -> p g d", d=w16)[:, :, co:co + wdt],
                                    in_=big[:, :bt * ncols]
                                    .rearrange("p (g d) -> p g d", d=ncols)[:, :, lo:hi])
                                co += wdt
                        co = 0
                        for (dt_, lo, hi, f16) in outs:
                            wdt = hi - lo
                            if f16:
                                nc.sync.dma_start(
                                    out=dt_.ap()[t0 * P:(t0 + bt) * P, :]
                                    .rearrange("(g p) d -> p g d", p=P),
                                    in_=cv[:, :bt * w16]
                                    .rearrange("p (g d) -> p g d", d=w16)[:, :, co:co + wdt])
                                co += wdt
                            else:
                                nc.sync.dma_start(
                                    out=dt_.ap()[t0 * P:(t0 + bt) * P, :]
                                    .rearrange("(g p) d -> p g d", p=P),
                                    in_=big[:, :bt * ncols]
                                    .rearrange("p (g d) -> p g d", d=ncols)[:, :, lo:hi])

                do(u_shards[l], UT,
                   wu_sb[:, l * 4 * D:(l + 1) * 4 * D], bu_sb[:, l * 4 * D:(l + 1) * 4 * D], 4 * D,
                   [(fs_tab[("rate", l)][0], 0, D, False),
                    (fs_tab[("tr", l)][0], D, 2 * D, False),
                    (fd_shard[("rb", l)], 2 * D, 3 * D, True),
                    (fd_shard[("tr", l)], 3 * D, 4 * D, True)])
                do(it_shards[l], IT,
                   wi_sb[:, l * 2 * D:(l + 1) * 2 * D], bi_sb[:, l * 2 * D:(l + 1) * 2 * D], 2 * D,
                   [(fd_shard[("rate", l)], 0, D, True),
                    (fs_tab[("rb", l)][0], D, 2 * D, False)])

            import os as _os3
            if _os3.environ.get("KNOAG") == "1":
                return
            for gname in ("rate", "rb", "tr"):
                ai, ao = fs_tab[(gname, l)]
                nc.gpsimd.collective_compute(
                    "AllGather", Alu.bypass, replica_groups=rg,
                    ins=[ai.ap()[:, :]], outs=[ao.ap()[:, :]])

        # ------------------------------------------------------------------
        def gats_phase(l, jobs):
            """Edge processing for all GATs of one layer, block-interleaved.

            jobs: list of (g, fs_table, fd_sh, out_tensor, resid_tab)."""
            import os as _os
            KG = int(_os.environ.get("KG", "9"))
            with ExitStack() as ctx:
                sb = ctx.enter_context(tc.tile_pool(name=f"egat{l}", bufs=2))
                ps_rep = ctx.enter_context(
                    tc.tile_pool(name=f"er{l}", bufs=3, space="PSUM"))
                ps_fde = ctx.enter_context(
                    tc.tile_pool(name=f"ef{l}", bufs=3, space="PSUM"))
                ps_acc = ctx.enter_context(
                    tc.tile_pool(name=f"ea{l}", bufs=2, space="PSUM"))

                def emit_block(job, wbi, w_base, g_base):
                    g, fs_table, fd_sh, out_tensor, resid_tab = job
                    gi = g_in[g.name]
                    a_l = a_sb[g.name][:, l * D:(l + 1) * D]
                    table_rows = fs_table.ap().shape[0]
                    K, Kb, nb = g.K, g.Kb, g.nb
                    G = wbi * K  # sub-tiles in block
                    # loads
                    idx_t = sb.tile([P, (g.WB * K * P) // 16], I16, tag="idx")
                    c0 = g_base * P // 16
                    nc.sync.dma_start(
                        out=idx_t[:, :G * P // 16],
                        in_=gi["idx"].ap()[:, c0:c0 + G * P // 16])
                    dlc_t = sb.tile([P, g.WB * K], F16, tag="dlc")
                    nc.sync.dma_start(out=dlc_t[:, :G],
                                      in_=gi["dlc"].ap()[:, g_base:g_base + G])
                    dlr_t = sb.tile([1, g.WB * K * P], F16, tag="dlr")
                    nc.sync.dma_start(
                        out=dlr_t[:, :G * P],
                        in_=gi["dlr"].ap()[:, g_base * P:(g_base + G) * P])
                    fd_t = sb.tile([P, g.WB * D], F16, tag="fd")
                    nc.sync.dma_start(
                        out=fd_t[:, :wbi * D],
                        in_=fd_sh.ap()[w_base * P:(w_base + wbi) * P, :]
                        .rearrange("(g p) d -> p g d", p=P))
                    if resid_tab is not None:
                        rs_t = sb.tile([P, g.WB * D], F32, tag="rs")
                        nc.sync.dma_start(
                            out=rs_t[:, :wbi * D],
                            in_=resid_tab.ap()[w_base * P:(w_base + wbi) * P, :]
                            .rearrange("(g p) d -> p g d", p=P))

                    fsg = sb.tile([P, g.WB * K * D], F32, tag="fsg")
                    # gathers per bank, round-robin over SWDGE queues
                    scol = 0
                    sg = 0
                    for b in (range(nb) if KG >= 1 else []):
                        ngb = wbi * Kb[b]          # sub-tiles for this bank
                        nidx = ngb * P
                        hi_row = min(table_rows, (b + 1) * BANK)
                        nc.gpsimd.dma_gather(
                            fsg[:, sg * D:(sg + ngb) * D]
                            .rearrange("p (g d) -> p g d", d=D),
                            fs_table.ap()[b * BANK:hi_row, :],
                            idx_t[:, scol:scol + nidx // 16],
                            nidx, nidx, D, single_packet=False,
                            queue_num=qctr[0] % 4)
                        qctr[0] += 1
                        scol += nidx // 16
                        sg += ngb
                    if KG <= 1:
                        out_t = sb.tile([P, g.WB * D], F32, tag="out")
                        nc.vector.tensor_copy(out=out_t[:, :wbi * D],
                                              in_=fsg[:, :wbi * D])
                        nc.sync.dma_start(
                            out=out_tensor.ap()[w_base * P:(w_base + wbi) * P, :]
                            .rearrange("(g p) d -> p g d", p=P),
                            in_=out_t[:, :wbi * D].rearrange("p (g d) -> p g d", d=D))
                        return
                    # acc psum for this block
                    acc = ps_acc.tile([P, g.WB * (D + 1)], F32, tag="acc",
                                      space="PSUM")

                    # sub-tile -> window map
                    win_of = []
                    for b in range(nb):
                        for wo in range(wbi):
                            win_of += [wo] * Kb[b]

                    # expansion one-hots for the whole block [P, G*P] fp16
                    # built per 4 sub-tiles (PSUM rep tile = 1 bank)
                    qt_big = sb.tile([P, g.WB * K * P], F16, tag="qtbig")
                    # scatter one-hots for the whole block: ONE DVE instr
                    if KG >= 4:
                        q_big = sb.tile([P, g.WB * K * P], F16, tag="qbig")
                        nc.vector.tensor_tensor(
                            out=q_big[:, :G * P]
                            .rearrange("p (g m) -> p g m", m=P),
                            in0=dlc_t[:, :G]
                            .rearrange("p (g m) -> p g m", m=1)
                            .to_broadcast([P, G, P]),
                            in1=im[:].rearrange("p (g m) -> p g m", g=1)
                            .to_broadcast([P, G, P]),
                            op=Alu.is_equal)

                    w_big = sb.tile([P, G * (D + 1)], F16, tag="wbig")
                    # X-stage groups of 8 sub-tiles
                    for x0 in range(0, G, 8):
                        xc = min(8, G - x0)
                        for r0 in range(x0, x0 + xc, 4):
                            rc = min(4, G - r0)
                            rep = ps_rep.tile([P, 4 * P], F32, tag="rep",
                                              space="PSUM")
                            nc.tensor.matmul(
                                rep[:, :rc * P], lhsT=onr[:],
                                rhs=dlr_t[:1, r0 * P:(r0 + rc) * P],
                                start=True, stop=True)
                            nc.vector.tensor_tensor(
                                out=qt_big[:, r0 * P:(r0 + rc) * P],
                                in0=ic[:].to_broadcast([P, rc * P]),
                                in1=rep[:, :rc * P],
                                op=Alu.is_equal)
                        fde = ps_fde.tile([P, 8 * D], F32, tag="fde", space="PSUM")
                        for j in range(xc):
                            gg = x0 + j
                            nc.tensor.matmul(
                                fde[:, j * D:(j + 1) * D],
                                lhsT=qt_big[:, gg * P:(gg + 1) * P],
                                rhs=fd_t[:, win_of[gg] * D:(win_of[gg] + 1) * D],
                                start=True, stop=True)
                        if KG == 2:
                            if x0 == 0:
                                out_t = sb.tile([P, g.WB * D], F32, tag="out")
                                nc.vector.tensor_copy(out=out_t[:, :wbi * D],
                                                      in_=fde[:, :wbi * D])
                            continue
                        # x = fs + fde ; leaky ; e ; z
                        x_t = sb.tile([P, 8 * D], F16, tag="x")
                        nc.vector.tensor_tensor(
                            out=x_t[:, :xc * D],
                            in0=fsg[:, x0 * D:(x0 + xc) * D],
                            in1=fde[:, :xc * D], op=Alu.add)
                        xs = sb.tile([P, 8 * D], F16, tag="xs")
                        nc.vector.tensor_scalar_mul(
                            xs[:, :xc * D], x_t[:, :xc * D], GAT_SLOPE)
                        xl = sb.tile([P, 8 * D], F16, tag="xl")
                        nc.vector.tensor_tensor(
                            out=xl[:, :xc * D], in0=x_t[:, :xc * D],
                            in1=xs[:, :xc * D], op=Alu.max)
                        xa = sb.tile([P, 8 * D], F32, tag="xa")
                        nc.vector.tensor_tensor(
                            out=xa[:, :xc * D], in0=xl[:, :xc * D],
                            in1=a_l.rearrange("p (g d) -> p g d", g=1)
                            .to_broadcast([P, xc, D]),
                            op=Alu.mult)
                        e8 = sb.tile([P, 8], F32, tag="e8")
                        nc.vector.reduce_sum(
                            out=e8[:, :xc],
                            in_=xa[:, :xc * D].rearrange("p (g d) -> p g d", d=D),
                            axis=mybir.AxisListType.X)
                        z8 = sb.tile([P, 8], F32, tag="z8")
                        nc.scalar.activation(z8[:, :xc], e8[:, :xc], Act.Exp)
                        nc.vector.tensor_tensor(
                            out=w_big[:, x0 * (D + 1):(x0 + xc) * (D + 1)]
                            .rearrange("p (g d) -> p g d", d=D + 1)[:, :, 0:D],
                            in0=fsg[:, x0 * D:(x0 + xc) * D]
                            .rearrange("p (g d) -> p g d", d=D),
                            in1=z8[:, :xc].rearrange("p (g d) -> p g d", d=1)
                            .to_broadcast([P, xc, D]),
                            op=Alu.mult)
                        nc.vector.tensor_copy(
                            out=w_big[:, x0 * (D + 1):(x0 + xc) * (D + 1)]
                            .rearrange("p (g d) -> p g d", d=D + 1)[:, :, D:D + 1],
                            in_=z8[:, :xc].rearrange("p (g d) -> p g d", d=1))
                        if KG == 3:
                            if x0 == 0:
                                out_t = sb.tile([P, g.WB * D], F32, tag="out")
                                nc.vector.tensor_copy(out=out_t[:, :wbi * D],
                                                      in_=w_big[:, :wbi * D])
                            continue

                    if KG >= 4:
                        # accumulation: window-major so PSUM groups in the
                        # shared bank are sequential, never interleaved
                        for wo in range(wbi):
                            subs = [gg for gg in range(G) if win_of[gg] == wo]
                            for si, gg in enumerate(subs):
                                nc.tensor.matmul(
                                    acc[:, wo * (D + 1):(wo + 1) * (D + 1)],
                                    lhsT=q_big[:, gg * P:(gg + 1) * P],
                                    rhs=w_big[:, gg * (D + 1):(gg + 1) * (D + 1)],
                                    start=(si == 0),
                                    stop=(si == len(subs) - 1))
                    if KG in (2, 3):
                        nc.sync.dma_start(
                            out=out_tensor.ap()[w_base * P:(w_base + wbi) * P, :]
                            .rearrange("(g p) d -> p g d", p=P),
                            in_=out_t[:, :wbi * D].rearrange("p (g d) -> p g d", d=D))
                        return
                    if KG == 4:
                        out_t = sb.tile([P, g.WB * D], F32, tag="out")
                        nc.vector.tensor_copy(
                            out=out_t[:, :wbi * D],
                            in_=acc[:, :g.WB * (D + 1)]
                            .rearrange("p (g d) -> p g d", d=D + 1)[:, :wbi, 0:D])
                        nc.sync.dma_start(
                            out=out_tensor.ap()[w_base * P:(w_base + wbi) * P, :]
                            .rearrange("(g p) d -> p g d", p=P),
                            in_=out_t[:, :wbi * D].rearrange("p (g d) -> p g d", d=D))
                        return
                    # divide + store (batched over the block's windows)
                    out_t = sb.tile([P, g.WB * D], F32, tag="out")
                    den = sb.tile([P, g.WB], F32, tag="den")
                    nc.vector.tensor_scalar_max(
                        den[:, :wbi].rearrange("p (w c) -> p w c", c=1),
                        acc[:, :wbi * (D + 1)]
                        .rearrange("p (w c) -> p w c", c=D + 1)[:, :, D:D + 1],
                        1e-30)
                    rec = sb.tile([P, g.WB], F32, tag="rec")
                    nc.vector.reciprocal(rec[:, :wbi], den[:, :wbi])
                    if resid_tab is None:
                        nc.vector.tensor_tensor(
                            out=out_t[:, :wbi * D]
                            .rearrange("p (w d) -> p w d", d=D),
                            in0=acc[:, :wbi * (D + 1)]
                            .rearrange("p (w c) -> p w c", c=D + 1)[:, :, 0:D],
                            in1=rec[:, :wbi]
                            .rearrange("p (w c) -> p w c", c=1)
                            .to_broadcast([P, wbi, D]),
                            op=Alu.mult)
                    else:
                        tmp = sb.tile([P, g.WB * D], F32, tag="dtmp")
                        nc.vector.tensor_tensor(
                            out=tmp[:, :wbi * D]
                            .rearrange("p (w d) -> p w d", d=D),
                            in0=acc[:, :wbi * (D + 1)]
                            .rearrange("p (w c) -> p w c", c=D + 1)[:, :, 0:D],
                            in1=rec[:, :wbi]
                            .rearrange("p (w c) -> p w c", c=1)
                            .to_broadcast([P, wbi, D]),
                            op=Alu.mult)
                        nc.vector.tensor_tensor(
                            out=out_t[:, :wbi * D], in0=tmp[:, :wbi * D],
                            in1=rs_t[:, :wbi * D], op=Alu.add)
                    nc.sync.dma_start(
                        out=out_tensor.ap()[w_base * P:(w_base + wbi) * P, :]
                        .rearrange("(g p) d -> p g d", p=P),
                        in_=out_t[:, :wbi * D].rearrange("p (g d) -> p g d", d=D))

                # interleave the jobs' blocks round-robin
                qctr = [0]
                cursors = []
                for job in jobs:
                    g = job[0]
                    w_base = 0
                    g_base = 0
                    blocks = []
                    for wbi in g.blocks:
                        blocks.append((wbi, w_base, g_base))
                        w_base += wbi
                        g_base += wbi * g.K
                    cursors.append(blocks)
                bi = 0
                while any(cursors):
                    for ji, job in enumerate(jobs):
                        if cursors[ji]:
                            emit_block(job, *cursors[ji].pop(0))
                    bi += 1

        # ------------------------------------------------------------------
        def epilogue_phase(l):
            with ExitStack() as ctx:
                sb = ctx.enter_context(tc.tile_pool(name=f"ep{l}", bufs=2))
                pst = ctx.enter_context(
                    tc.tile_pool(name=f"ept{l}", bufs=2, space="PSUM"))
                psm = ctx.enter_context(
                    tc.tile_pool(name=f"epm{l}", bufs=4, space="PSUM"))
                BT = 8
                w1_l = w1_sb[:, l * 2 * D:(l + 1) * 2 * D]
                b1_l = b1_sb[:, l * 2 * D:(l + 1) * 2 * D]
                w2_l = w2_sb[:, l * 2 * D:(l + 1) * 2 * D]
                b2_l = b2_sb[:, l * 2:(l + 1) * 2]
                for t0 in range(0, UT, BT):
                    bt = min(BT, UT - t0)
                    rows = slice(t0 * P, (t0 + bt) * P)
                    ut = sb.tile([P, BT * D], F32, tag="eu")
                    nc.sync.dma_start(
                        out=ut[:, :bt * D].rearrange("p (g d) -> p g d", d=D),
                        in_=u_shards[l].ap()[rows, :].rearrange("(g p) d -> p g d", p=P))
                    pt = sb.tile([P, BT * D], F32, tag="epp")
                    nc.sync.dma_start(
                        out=pt[:, :bt * D].rearrange("p (g d) -> p g d", d=D),
                        in_=p_sh.ap()[rows, :].rearrange("(g p) d -> p g d", p=P))
                    qt_ = sb.tile([P, BT * D], F32, tag="epq")
                    nc.sync.dma_start(
                        out=qt_[:, :bt * D].rearrange("p (g d) -> p g d", d=D),
                        in_=q_sh.ap()[rows, :].rearrange("(g p) d -> p g d", p=P))
                    ot = sb.tile([P, BT * D], F32, tag="eo")
                    for k in range(bt):
                        ts = []
                        for srcp in (ut, pt, qt_):
                            tp = pst.tile([D, P], F32, tag="etp")
                            nc.tensor.transpose(
                                out=tp[:], in_=srcp[:, k * D:(k + 1) * D],
                                identity=ident[:])
                            ts.append(tp)
                        ct_inf = sb.tile([2 * D, P], F32, tag="ecti")
                        nc.vector.tensor_copy(out=ct_inf[0:D, :], in_=ts[0][:])
                        nc.vector.tensor_copy(out=ct_inf[D:2 * D, :], in_=ts[1][:])
                        ct_int = sb.tile([2 * D, P], F32, tag="ectj")
                        nc.vector.tensor_copy(out=ct_int[0:D, :], in_=ts[0][:])
                        nc.vector.tensor_copy(out=ct_int[D:2 * D, :], in_=ts[2][:])
                        s2 = []
                        for ci, (ct, col) in enumerate(((ct_inf, 0), (ct_int, 1))):
                            mm = psm.tile([P, D], F32, tag="emm")
                            nc.tensor.matmul(
                                mm[:], lhsT=ct[:],
                                rhs=w1_l[:, col * D:(col + 1) * D],
                                start=True, stop=True)
                            s1 = sb.tile([P, D], F32, tag="es1")
                            nc.vector.tensor_tensor(
                                out=s1[:], in0=mm[:],
                                in1=b1_l[:, col * D:(col + 1) * D], op=Alu.add)
                            s1s = sb.tile([P, D], F32, tag="es1s")
                            nc.vector.tensor_scalar_mul(s1s[:], s1[:], MLP_SLOPE)
                            s1l = sb.tile([P, D], F32, tag="es1l")
                            nc.vector.tensor_tensor(
                                out=s1l[:], in0=s1[:], in1=s1s[:], op=Alu.max)
                            xw = sb.tile([P, D], F32, tag="exw")
                            nc.vector.tensor_tensor(
                                out=xw[:], in0=s1l[:],
                                in1=w2_l[:, col * D:(col + 1) * D], op=Alu.mult)
                            sv0 = sb.tile([P, 1], F32, tag="esv0")
                            nc.vector.reduce_sum(out=sv0[:], in_=xw[:],
                                                 axis=mybir.AxisListType.X)
                            sv = sb.tile([P, 1], F32, tag="esv")
                            nc.vector.tensor_tensor(
                                out=sv[:], in0=sv0[:],
                                in1=b2_l[:, col:col + 1], op=Alu.add)
                            svs = sb.tile([P, 1], F32, tag="esvs")
                            nc.vector.tensor_scalar_mul(svs[:], sv[:], MLP_SLOPE)
                            svl = sb.tile([P, 1], F32, tag="esvl")
                            nc.vector.tensor_tensor(
                                out=svl[:], in0=sv[:], in1=svs[:], op=Alu.max)
                            s2.append(svl)
                        dg = sb.tile([P, 1], F32, tag="edg")
                        nc.vector.tensor_tensor(
                            out=dg[:], in0=s2[0][:], in1=s2[1][:], op=Alu.subtract)
                        g0 = sb.tile([P, 1], F32, tag="eg0")
                        nc.scalar.activation(g0[:], dg[:], Act.Sigmoid)
                        pk = pt[:, k * D:(k + 1) * D]
                        qk = qt_[:, k * D:(k + 1) * D]
                        uk = ut[:, k * D:(k + 1) * D]
                        pq = sb.tile([P, D], F32, tag="epq2")
                        nc.vector.tensor_tensor(out=pq[:], in0=pk, in1=qk,
                                                op=Alu.subtract)
                        gpq = sb.tile([P, D], F32, tag="egpq")
                        nc.vector.tensor_scalar_mul(gpq[:], pq[:], g0[:, :1])
                        uq = sb.tile([P, D], F32, tag="euq")
                        nc.vector.tensor_tensor(out=uq[:], in0=uk, in1=qk,
                                                op=Alu.add)
                        nc.vector.tensor_tensor(
                            out=ot[:, k * D:(k + 1) * D], in0=uq[:], in1=gpq[:],
                            op=Alu.add)
                    nc.sync.dma_start(
                        out=u_shards[l + 1].ap()[rows, :]
                        .rearrange("(g p) d -> p g d", p=P),
                        in_=ot[:, :bt * D].rearrange("p (g d) -> p g d", d=D))

        # ------------------------------------------------------------------
        def hu_pred_phase():
            """Assemble hu/hi fp16 tables, then per-edge gathers + dots.

            One pool set so pred's hu-side gathers (ready once hu_t is
            written) overlap the hi_t build."""
            with ExitStack() as ctx:
                sb = ctx.enter_context(tc.tile_pool(name="hup", bufs=2))
                BT = 8
                for tabs, out_tab, n_tiles in ((u_tabs, hu_t, UPAD // P),
                                               (it_tabs, hi_t, IPAD // P)):
                    for t0 in range(0, n_tiles, BT):
                        bt = min(BT, n_tiles - t0)
                        rows = slice(t0 * P, (t0 + bt) * P)
                        big = sb.tile([P, BT * PD], F16, tag="hbig")
                        nc.vector.memset(big[:], 0)
                        for li, tab in enumerate(tabs):
                            ld = sb.tile([P, BT * D], F32, tag="hld")
                            nc.sync.dma_start(
                                out=ld[:, :bt * D].rearrange("p (g d) -> p g d", d=D),
                                in_=tab.ap()[rows, :]
                                .rearrange("(g p) d -> p g d", p=P))
                            nc.vector.tensor_copy(
                                out=big[:, :bt * PD]
                                .rearrange("p (g d) -> p g d", d=PD)
                                [:, :, li * D:(li + 1) * D],
                                in_=ld[:, :bt * D]
                                .rearrange("p (g d) -> p g d", d=D))
                        nc.sync.dma_start(
                            out=out_tab.ap()[rows, :]
                            .rearrange("(g p) d -> p g d", p=P),
                            in_=big[:, :bt * PD].rearrange("p (g d) -> p g d", d=PD))

                G = pred.G_blk
                for bi in range(pred.n_blocks):
                    hu_g = sb.tile([P, G * PD], F16, tag="phu")
                    hi_g = sb.tile([P, G * PD], F16, tag="phi")
                    iu_t = sb.tile([P, G * P // 16], I16, tag="piu")
                    c0 = bi * G * P // 16
                    nc.sync.dma_start(out=iu_t[:],
                                      in_=pidxu.ap()[:, c0:c0 + G * P // 16])
                    ii_t = sb.tile([P, G * P // 16], I16, tag="pii")
                    nc.sync.dma_start(out=ii_t[:],
                                      in_=pidxi.ap()[:, c0:c0 + G * P // 16])
                    # hu gathers: per user bank (spans its item-bank pairs)
                    sg = 0
                    scol = 0
                    qn = 0
                    for u_ in range(pred.nbu):
                        ngb = sum(pred.Kp[(u_, i_)] for i_ in range(pred.nbi))
                        nidx = ngb * P
                        hi_row = min(hu_t.ap().shape[0], (u_ + 1) * BANK)
                        nc.gpsimd.dma_gather(
                            hu_g[:, sg * PD:(sg + ngb) * PD]
                            .rearrange("p (g d) -> p g d", d=PD),
                            hu_t.ap()[u_ * BANK:hi_row, :],
                            iu_t[:, scol:scol + nidx // 16],
                            nidx, nidx, PD, single_packet=False,
                            queue_num=qn % 4)
                        qn += 1
                        sg += ngb
                        scol += nidx // 16
                    # hi gathers: per pair
                    sg = 0
                    scol = 0
                    for u_ in range(pred.nbu):
                        for i_ in range(pred.nbi):
                            ngb = pred.Kp[(u_, i_)]
                            nidx = ngb * P
                            hi_row = min(hi_t.ap().shape[0], (i_ + 1) * BANK)
                            nc.gpsimd.dma_gather(
                                hi_g[:, sg * PD:(sg + ngb) * PD]
                                .rearrange("p (g d) -> p g d", d=PD),
                                hi_t.ap()[i_ * BANK:hi_row, :],
                                ii_t[:, scol:scol + nidx // 16],
                                nidx, nidx, PD, single_packet=False,
                                queue_num=qn % 4)
                            qn += 1
                            sg += ngb
                            scol += nidx // 16
                    # dots (batched mult + per-group reduce)
                    dt_ = sb.tile([P, G], F32, tag="pdot")
                    for x0 in range(0, G, 8):
                        xc = min(8, G - x0)
                        prod = sb.tile([P, 8 * PD], F32, tag="pprod")
                        nc.vector.tensor_tensor(
                            out=prod[:, :xc * PD],
                            in0=hu_g[:, x0 * PD:(x0 + xc) * PD],
                            in1=hi_g[:, x0 * PD:(x0 + xc) * PD], op=Alu.mult)
                        nc.vector.reduce_sum(
                            out=dt_[:, x0:x0 + xc],
                            in_=prod[:, :xc * PD]
                            .rearrange("p (g d) -> p g d", d=PD),
                            axis=mybir.AxisListType.X)
                    nc.sync.dma_start(out=pred_out.ap()[:, bi * G:(bi + 1) * G],
                                      in_=dt_[:])

        # ------------------------------------------------------------------
        phase_order = []
        for l in range(L):
            phase_order += [f"proj{l}", f"gats{l}", f"agi{l}", f"epi{l}",
                            f"ag{l}"]
        phase_order += ["pred"]

        def run_until():
            if kphase == "null":
                return
            for ph in phase_order:
                l = int(ph[-1]) if ph[-1].isdigit() else 0
                if ph.startswith("proj"):
                    proj_phase(l)
                elif ph.startswith("gats"):
                    gats_phase(l, [
                        (rate, fs_tab[("rate", l)][1], fd_shard[("rate", l)],
                         it_shards[l + 1], it_shards[l]),
                        (rb, fs_tab[("rb", l)][1], fd_shard[("rb", l)],
                         q_sh, None),
                        (tr, fs_tab[("tr", l)][1], fd_shard[("tr", l)],
                         p_sh, None),
                    ])
                elif ph.startswith("epi"):
                    epilogue_phase(l)
                elif ph.startswith("agi"):
                    # item-table AllGather: input ready at end of gats, so
                    # this overlaps the epilogue on the collective engine
                    nc.gpsimd.collective_compute(
                        "AllGather", Alu.bypass, replica_groups=rg,
                        ins=[it_shards[l + 1].ap()[:, :]],
                        outs=[it_tabs[l + 1].ap()[:, :]])
                elif ph.startswith("ag"):
                    nc.gpsimd.collective_compute(
                        "AllGather", Alu.bypass, replica_groups=rg,
                        ins=[u_shards[l + 1].ap()[:, :]],
                        outs=[u_tabs[l + 1].ap()[:, :]])
                elif ph == "pred":
                    hu_pred_phase()
                if ph == kphase:
                    return

        run_until()
        if dbg_out is not None:
            dbg_tensors = dict(
                q_sh=q_sh, p_sh=p_sh, hu=hu_t, hi=hi_t,
                **{f"u_shard{i}": t for i, t in enumerate(u_shards)},
                **{f"it_shard{i}": t for i, t in enumerate(it_shards)},
                **{f"u_tab{i}": t for i, t in enumerate(u_tabs)},
                **{f"it_tab{i}": t for i, t in enumerate(it_tabs)},
                **{f"fs_{nm}{l}": fs_tab[(nm, l)][1] for nm in ("rate", "rb", "tr")
                   for l in range(L)},
                **{f"fsin_{nm}{l}": fs_tab[(nm, l)][0] for nm in ("rate", "rb", "tr")
                   for l in range(L)},
                **{f"fd_{nm}{l}": fd_shard[(nm, l)] for nm in ("rate", "rb", "tr")
                   for l in range(L)},
            )
            src_t = dbg_tensors[dbg_spec[0]]
            sdt = src_t.ap().dtype
            with ExitStack() as ctx:
                sbd = ctx.enter_context(tc.tile_pool(name="dbg", bufs=2))
                rows, cols = dbg_spec[1], dbg_spec[2]
                for r0 in range(0, rows, P):
                    rc = min(P, rows - r0)
                    t_ = sbd.tile([P, cols], sdt, tag="dbg")
                    nc.sync.dma_start(out=t_[:rc, :],
                                      in_=src_t.ap()[r0:r0 + rc, :])
                    if sdt != F32:
                        t2 = sbd.tile([P, cols], F32, tag="dbg2")
                        nc.vector.tensor_copy(out=t2[:rc, :], in_=t_[:rc, :])
                        t_ = t2
                    nc.sync.dma_start(out=dbg_out.ap()[r0:r0 + rc, :],
                                      in_=t_[:rc, :])

    nc.compile()
    return nc


# ---------------------------------------------------------------------------
# entry point
# ---------------------------------------------------------------------------

def _pad_rows(a, rows):
    out = np.zeros((rows, a.shape[1]), dtype=a.dtype)
    out[:a.shape[0]] = a
    return out


def kernel(**inputs):
    U, D = inputs["user_emb"].shape
    I = inputs["item_emb"].shape[0]
    L = inputs["rate_Ws"].shape[0]
    UT = _ceil(_ceil(U, P), N_CORES)
    IT = _ceil(_ceil(I, P), N_CORES)
    US, IS = UT * P, IT * P
    UPAD, IPAD = US * N_CORES, IS * N_CORES
    PD = _ceil(D * (L + 1), P) * P if D * (L + 1) % P else D * (L + 1)
    # gather elem size must be a multiple of 256 bytes -> PD*2 % 256 == 0
    PD = _ceil(D * (L + 1) * 2, 256) * 128

    rate_src = np.asarray(inputs["rate_src"])
    rate_dst = np.asarray(inputs["rate_dst"])
    trust_src = np.asarray(inputs["trust_src"])
    trust_dst = np.asarray(inputs["trust_dst"])

    rate = GatStruct("rate", rate_src, rate_dst, UPAD, IT)
    rb = GatStruct("rb", rate_dst, rate_src, IPAD, UT)
    tr = GatStruct("tr", trust_src, trust_dst, UPAD, UT)

    pos_src = np.asarray(inputs["pos_src"])
    pos_dst = np.asarray(inputs["pos_dst"])
    neg_src = np.asarray(inputs["neg_src"])
    neg_dst = np.asarray(inputs["neg_dst"])
    psrc = np.concatenate([pos_src, neg_src])
    pdst = np.concatenate([pos_dst, neg_dst])
    pred = PredStruct(psrc, pdst, UPAD, IPAD, block_edges=9216)

    import os
    hp = dict(U=U, I=I, D=D, L=L, UT=UT, IT=IT, PD=PD,
              rate=rate, rb=rb, tr=tr, pred=pred)
    print(f"[kernel] struct: rate K={rate.K} Kb={rate.Kb} WB={rate.WB} blocks={len(rate.blocks)}; "
          f"rb K={rb.K} WB={rb.WB} blocks={len(rb.blocks)}; "
          f"tr K={tr.K} WB={tr.WB} blocks={len(tr.blocks)}; "
          f"pred G_blk={pred.G_blk} blocks={pred.n_blocks}")
    kdbg = os.environ.get("KDBG")
    if kdbg:
        shp = {}
        for i in range(L + 1):
            shp[f"u_shard{i}"] = (US, D); shp[f"it_shard{i}"] = (IS, D)
            shp[f"u_tab{i}"] = (UPAD, D); shp[f"it_tab{i}"] = (IPAD, D)
        for l in range(L):
            shp[f"fs_rate{l}"] = (UPAD, D); shp[f"fs_tr{l}"] = (UPAD, D)
            shp[f"fs_rb{l}"] = (IPAD, D)
            shp[f"fsin_rate{l}"] = (US, D); shp[f"fsin_tr{l}"] = (US, D)
            shp[f"fsin_rb{l}"] = (IS, D)
            shp[f"fd_rate{l}"] = (IS, D); shp[f"fd_rb{l}"] = (US, D)
            shp[f"fd_tr{l}"] = (US, D)
        shp["q_sh"] = (US, D); shp["p_sh"] = (US, D)
        shp["hu"] = (UPAD, PD); shp["hi"] = (IPAD, PD)
        hp["dbg_spec"] = (kdbg, *shp[kdbg])

    t_b = __import__("time").time()
    nc = build_program(hp)
    print(f"[kernel] build+compile: {__import__('time').time() - t_b:.1f}s")

    # ---- inputs ----
    f16 = NPF16
    ue_pad = _pad_rows(inputs["user_emb"].astype(np.float32), UPAD)
    ie_pad = _pad_rows(inputs["item_emb"].astype(np.float32), IPAD)
    wu = np.concatenate([
        np.concatenate([inputs["rate_Ws"][l], inputs["tr_Ws"][l],
                        inputs["rb_Wd"][l], inputs["tr_Wd"][l]], axis=1)
        for l in range(L)], axis=1).astype(np.float32)
    bu = np.concatenate([
        np.tile(np.concatenate([inputs["rate_bs"][l], inputs["tr_bs"][l],
                                inputs["rb_bd"][l], inputs["tr_bd"][l]])[None, :],
                (P, 1))
        for l in range(L)], axis=1).astype(np.float32)
    wi = np.concatenate([
        np.concatenate([inputs["rate_Wd"][l], inputs["rb_Ws"][l]], axis=1)
        for l in range(L)], axis=1).astype(np.float32)
    bi_ = np.concatenate([
        np.tile(np.concatenate([inputs["rate_bd"][l], inputs["rb_bs"][l]])[None, :],
                (P, 1))
        for l in range(L)], axis=1).astype(np.float32)
    a_arrs = {}
    for nm in ("rate", "rb", "tr"):
        a_arrs[nm] = np.concatenate([
            np.tile(np.asarray(inputs[f"{nm}_a"][l])[None, :], (P, 1))
            for l in range(L)], axis=1).astype(np.float32)
    w1 = np.concatenate([
        np.concatenate([inputs["inf_W1"][l], inputs["int_W1"][l]], axis=1)
        for l in range(L)], axis=1).astype(np.float32)
    b1 = np.concatenate([
        np.tile(np.concatenate([inputs["inf_b1"][l], inputs["int_b1"][l]])[None, :],
                (P, 1))
        for l in range(L)], axis=1).astype(np.float32)
    w2 = np.concatenate([
        np.tile(np.concatenate([inputs["inf_W2"][l][:, 0],
                                inputs["int_W2"][l][:, 0]])[None, :], (P, 1))
        for l in range(L)], axis=1).astype(np.float32)
    b2 = np.concatenate([
        np.tile(np.array([[inputs["inf_b2"][l][0], inputs["int_b2"][l][0]]],
                         dtype=np.float32), (P, 1))
        for l in range(L)], axis=1).astype(np.float32)
    iota = np.arange(P, dtype=np.float32)
    iota_m = np.tile(iota[None, :], (P, 1)).astype(f16)
    iota_c = iota[:, None].astype(f16)
    ones_r = np.ones((1, P), dtype=f16)

    in_maps = []
    for c in range(N_CORES):
        m = {
            "user_emb": ue_pad, "item_emb": ie_pad,
            "u_shard0": ue_pad[c * US:(c + 1) * US],
            "it_shard0": ie_pad[c * IS:(c + 1) * IS],
            "wu": wu, "bu": bu, "wi": wi, "bi": bi_,
            "a_rate": a_arrs["rate"], "a_rb": a_arrs["rb"], "a_tr": a_arrs["tr"],
            "w1": w1, "b1": b1, "w2": w2, "b2": b2,
            "iota_m": iota_m, "iota_c": iota_c, "ones_r": ones_r,
            "pred_idxu": pred.idxu[c], "pred_idxi": pred.idxi[c],
        }
        for g in (rate, rb, tr):
            m[f"{g.name}_idx"] = g.idx16[c]
            m[f"{g.name}_dlc"] = g.dlc[c]
            m[f"{g.name}_dlr"] = g.dlr[c]
        in_maps.append(m)

    trace = os.environ.get("KTRACE") == "1"
    global LAST_RES, LAST_HP, LAST_EXEC_NS
    if os.environ.get("KSKIPRUN") == "1":
        class _FakeRes:
            results = [{"pred_out": np.zeros((P, pred.G_total), np.float32)}
                       for _ in range(N_CORES)]
            exec_time_ns = None
        res = _FakeRes()
    else:
        t_run = __import__("time").time()
        res = run_bass_kernel_spmd(nc, in_maps, core_ids=list(range(N_CORES)),
                                   trace=trace)
        print(f"[kernel] device run wall: {__import__('time').time() - t_run:.1f}s")
    LAST_RES, LAST_HP, LAST_EXEC_NS = res, hp, res.exec_time_ns
    if os.environ.get("KBENCH") == "1":
        tmin = bench_pjrt(nc, in_maps, iters=int(os.environ.get("KBENCH_ITERS", "4")))
        LAST_EXEC_NS = int(tmin * 1e9)

    # ---- assemble outputs ----
    E = len(psrc)
    out = np.zeros((E,), dtype=np.float32)
    for c in range(N_CORES):
        vals = res.results[c]["pred_out"]  # [128, G_total]
        smap = pred.slotmap[c]
        flat = vals.T.reshape(-1)          # slot s = (p, g) -> g*128 + p? no:
        # slot i = (partition i%128, group i//128) => value at vals[i%128, i//128]
        gidx = np.arange(len(smap))
        v = vals[gidx % P, gidx // P]
        ok = smap >= 0
        out[smap[ok]] = v[ok]
    pos = out[:len(pos_src)].reshape(-1, 1)
    neg = out[len(pos_src):].reshape(-1, 1)
    return pos, neg

